# revision 47
# baseline (speedup 1.0000x reference)
"""GATv2 (3-layer, heads=4/4/1) full-graph kernel for 8 Trainium2 NeuronCores.

Contract: kernel(**inputs) takes the FULL unsharded inputs (as produced by
setup_inputs()) and returns the FULL [64, 64] float32 output.

v4 design (vs. v3 baseline at 1.90 ms):
- Layer-0 phase A is REPLICATED: every core computes xl0 for all 50176 nodes
  straight from the (fully available) input x and writes it to local DRAM.
  The 251 us layer-0 feature AllGather is gone entirely.
- Layers 1-2 exchange xl via CHUNKED AllGathers overlapped with compute:
  phase A(l+1) for a chunk of own tiles runs as soon as phase B(l) finishes
  those tiles, and the chunk's AllGather fires immediately, running on the
  collective cores while phase B(l) continues on later chunks.  xl_full rows
  are laid out (chunk, core, row)-major so every AllGather lands in a
  contiguous slice with IR identical on all cores.
- Gathers issue one dma_gather per (group, stream) (up to 3840 indices per
  call, 8192-descriptor SWDGE ring) to amortize the ~1 us fixed SWDGE cost.
- Pad targets (node ids >= N) get a fake self-loop so every target has a
  nonzero softmax denominator; per-tile normalization is then a single DVE
  divide (no max/reciprocal dance, no NaNs reaching the pool matmul).
- Per-edge xr is expanded on the PE from SBUF-resident xr tiles via host-built
  fp8 one-hot matrices (oh: [lane,tgt], ohT: [tgt,lane]); gathered xl rows are
  accumulated into the same PSUM via an identity matmul; leaky-relu applied
  straight from PSUM on ACT; scores via DVE mult + halving-tree; softmax
  without max-shift (scores empirically in [-8, 7]); scatter-sum + denominators
  via fp8 one-hot matmul into PSUM; global-mean-pool partials via PE, summed
  and divided on the host.
"""
import os
import numpy as np
import ml_dtypes

import concourse.bacc as bacc
import concourse.mybir as mybir
import concourse.tile as tile
from concourse._compat import get_trn_type
from concourse.bass_utils import run_bass_kernel_spmd

f16 = mybir.dt.float16
f32 = mybir.dt.float32
f8 = mybir.dt.float8e4
i16 = mybir.dt.int16
f8np = ml_dtypes.float8_e4m3

P = 128
N = 50000
E = 800000
NP_ = 50176            # padded nodes = 392 * 128
NT = NP_ // P          # 392 global tiles
CORES = 8
NTC = NT // CORES      # 49 tiles per core
NC_NODES = NTC * P     # 6272 nodes per core
HALF = NP_ // 2        # 25088 rows per shared half
G_GRAPHS = 64
NEG = 0.2
GROUP = 3              # tiles per gather/compute group
BATCH = 8              # chunks per PSUM u-batch
H_L = [4, 4, 1]
W_L = [128, 128, 64]   # xl/value width per layer
CHUNKS = [(0, 17), (17, 16), (33, 16)]  # (tt0, sz) ag-pipeline chunks of NTC

_CACHE = {}


def _pack_idx_image(seq):
    """int16 index sequence -> gather SBUF image [128, len/16]."""
    n = len(seq)
    assert n % 16 == 0
    img = np.asarray(seq, np.int16).reshape(n // 16, 16).T
    return np.tile(img, (8, 1))


def _chunk_groups(tt0, sz):
    out = []
    gi = tt0
    while gi < tt0 + sz:
        out.append((gi, min(GROUP, tt0 + sz - gi)))
        gi += GROUP
    return out


def _group_list():
    out = []
    for tt0, sz in CHUNKS:
        out.extend(_chunk_groups(tt0, sz))
    return out


def _tile_slot():
    """global tile t -> slot in the (chunk, core, row) xl_full layout."""
    slot = np.empty(NT, np.int64)
    for c in range(CORES):
        for tt0, sz in CHUNKS:
            for i in range(sz):
                slot[NTC * c + tt0 + i] = CORES * tt0 + c * sz + i
    return slot


def _pack_perm(h, c):
    """column permutation: packed[cw*h_n + hh] = natural[hh*c + cw]."""
    perm = np.empty(h * c, np.int64)
    for cw in range(c):
        for hh in range(h):
            perm[cw * h + hh] = hh * c + cw
    return perm


def _balance_perm(edge_index):
    """Relabel real nodes so every 128-node tile has near-equal in-degree.
    Returns perm[orig] -> new position (pads N..NP_ stay in place)."""
    import heapq
    deg = np.bincount(edge_index[1].astype(np.int64), minlength=N) + 1
    order = np.argsort(-deg, kind="stable")
    nfull = N // P                      # 390 full tiles
    caps = [P] * nfull + [N - nfull * P]  # tile 390 gets the remainder
    heap = [(0, b) for b in range(len(caps))]
    heapq.heapify(heap)
    fill = [0] * len(caps)
    perm = np.empty(N, np.int64)
    for v in order:
        while True:
            s, b = heapq.heappop(heap)
            if fill[b] < caps[b]:
                break
        perm[v] = b * P + fill[b]
        fill[b] += 1
        if fill[b] < caps[b]:
            heapq.heappush(heap, (s + int(deg[v]), b))
    return perm


def _preprocess(x, edge_index, batch, params):
    nperm = _balance_perm(edge_index)
    loops = np.arange(N, dtype=np.int64)
    pads = np.arange(N, NP_, dtype=np.int64)   # fake self-loops on pad targets
    src = np.concatenate([nperm[edge_index[0].astype(np.int64)], nperm[loops],
                          pads])
    tgt = np.concatenate([nperm[edge_index[1].astype(np.int64)], nperm[loops],
                          pads])
    order = np.argsort(tgt, kind="stable")
    srcs, tgts = src[order], tgt[order]

    slot = _tile_slot()
    src_row = slot[srcs // P] * P + srcs % P   # permuted xl_full row per edge
    islo = src_row < HALF

    bounds = np.searchsorted(tgts, np.arange(0, NP_ + 1, P))
    nlo = np.empty(NT, np.int64)
    nhi = np.empty(NT, np.int64)
    for t in range(NT):
        s, e = bounds[t], bounds[t + 1]
        nlo[t] = int(islo[s:e].sum())
        nhi[t] = (e - s) - nlo[t]
    # per-core-tile-slot chunk counts (max over cores, static across SPMD IR)
    chs_lo = tuple(
        int(max(1, -(-nlo[tt::NTC].max() // P))) for tt in range(NTC))
    chs_hi = tuple(
        int(max(1, -(-nhi[tt::NTC].max() // P))) for tt in range(NTC))

    # per-layer packed weights / attention
    wlrs, att_reps = [], []
    prev_perm = None  # input-feature permutation (packing of previous layer)
    for li, (Wl, Wr, att) in enumerate(params):
        h, c = att.shape
        hc = h * c
        Wl = np.asarray(Wl, np.float32)
        Wr = np.asarray(Wr, np.float32)
        if prev_perm is not None:
            Wl = Wl[prev_perm]
            Wr = Wr[prev_perm]
        if li < 2:
            perm = _pack_perm(h, c)
            Wl = Wl[:, perm]
            Wr = Wr[:, perm]
            att_flat = np.asarray(att, np.float32).reshape(-1)[perm]
            prev_perm = perm
        else:
            att_flat = np.asarray(att, np.float32).reshape(-1)
            prev_perm = None
        wlr = np.zeros((P, 256), np.float16)
        wlr[: Wl.shape[0], :hc] = Wl.astype(np.float16)
        wlr[: Wr.shape[0], 128 : 128 + hc] = Wr.astype(np.float16)
        wlrs.append(wlr)
        af = np.zeros(P, np.float16)
        af[:hc] = att_flat.astype(np.float16)
        att_reps.append(np.tile(af[None, :], (P, 1)))

    ident = np.eye(P).astype(f8np)

    x_pad = np.zeros((NP_, P), np.float32)
    x_pad[nperm] = np.asarray(x, np.float32)   # rows at balanced positions
    # x columns in slot-major (permuted) order, shared by all cores
    slot_tile = np.empty(NT, np.int64)
    slot_tile[slot] = np.arange(NT)
    xTp = np.empty((P, NP_), np.float16)
    for s in range(NT):
        t = slot_tile[s]
        xTp[:, s * P:(s + 1) * P] = x_pad[t * P:(t + 1) * P].astype(np.float16).T

    grp_list = _group_list()

    # graph id per NEW position (pads -> 0, masked out by valid)
    batch_perm = np.zeros(NP_, np.int64)
    batch_perm[nperm] = np.asarray(batch, np.int64)
    valid_perm = np.zeros(NP_, bool)
    valid_perm[nperm] = True

    in_maps = []
    for c in range(CORES):
        t0 = c * NTC
        base = t0 * P
        # per-(tile, stream) slot tables, padded to chs_*[tt]*128
        xlo = [np.zeros(chs_lo[tt] * P, np.int64) for tt in range(NTC)]
        xhi = [np.zeros(chs_hi[tt] * P, np.int64) for tt in range(NTC)]
        tl_lo = [np.full(chs_lo[tt] * P, -1, np.int64) for tt in range(NTC)]
        tl_hi = [np.full(chs_hi[tt] * P, -1, np.int64) for tt in range(NTC)]
        for tt in range(NTC):
            t = t0 + tt
            s, e = bounds[t], bounds[t + 1]
            sl = tgts[s:e] - t * P
            sp = src_row[s:e]
            lo_mask = islo[s:e]
            k = int(lo_mask.sum()); k2 = (e - s) - k
            xlo[tt][:k] = sp[lo_mask]
            tl_lo[tt][:k] = sl[lo_mask]
            xhi[tt][:k2] = sp[~lo_mask] - HALF
            tl_hi[tt][:k2] = sl[~lo_mask]

        # group-stream-major chunk columns
        lo_imgs, hi_imgs = [], []
        oh_cols, ohT_cols = [], []
        for gi, g in grp_list:
            lo_seq = np.concatenate(xlo[gi:gi + g])
            hi_seq = np.concatenate(xhi[gi:gi + g])
            lo_imgs.append(_pack_idx_image(lo_seq))
            hi_imgs.append(_pack_idx_image(hi_seq))
            tl_seq = np.concatenate(tl_lo[gi:gi + g] + tl_hi[gi:gi + g])
            nch2 = len(tl_seq) // P
            tl_mat = tl_seq.reshape(nch2, P)          # [chunk, lane] -> tloc
            oh = np.zeros((P, nch2, P), f8np)         # [lane, chunk, tgt]
            ohT = np.zeros((P, nch2, P), f8np)        # [tgt, chunk, lane]
            ch_i, ln_i = np.nonzero(tl_mat >= 0)
            tl_v = tl_mat[ch_i, ln_i]
            oh[ln_i, ch_i, tl_v] = 1.0
            ohT[tl_v, ch_i, ln_i] = 1.0
            oh_cols.append(oh)
            ohT_cols.append(ohT)

        # pooling one-hot [128, NTC, 64]
        pool = np.zeros((P, NTC, G_GRAPHS), np.float16)
        for tt in range(NTC):
            gn = base + tt * P + np.arange(P)
            valid = valid_perm[gn]
            pool[valid, tt, batch_perm[gn[valid]]] = 1.0

        # own-shard x columns (natural tt order) for the layer-0 xr pass
        xr0T = np.ascontiguousarray(
            x_pad[base:base + NC_NODES].astype(np.float16).T)

        in_maps.append({
            "x0T": xTp,
            "xr0T": xr0T,
            "xlidxlo": np.concatenate(lo_imgs, axis=1),
            "xlidxhi": np.concatenate(hi_imgs, axis=1),
            "oh": np.concatenate(oh_cols, axis=1),
            "ohT": np.concatenate(ohT_cols, axis=1),
            "ident": ident,
            "attr0": att_reps[0], "attr1": att_reps[1], "attr2": att_reps[2],
            "wlr0": wlrs[0], "wlr1": wlrs[1], "wlr2": wlrs[2],
            "pooloh": pool,
        })

    return dict(chs_lo=chs_lo, chs_hi=chs_hi), in_maps


def _build(meta):
    chs_lo, chs_hi = meta["chs_lo"], meta["chs_hi"]
    NIL = sum(chs_lo)   # lo chunks per core
    NIH = sum(chs_hi)
    NCH = NIL + NIH     # total chunk columns per core
    MCH = max(max(chs_lo), max(chs_hi))
    nc = bacc.Bacc(
        get_trn_type() or "TRN2",
        target_bir_lowering=False,
        debug=False,
        num_devices=CORES,
        dynamic_dma_scratch_size=32768,   # 2048-descriptor SWDGE ring
    )
    inp = {}
    for name, shape, dt in [
        ("x0T", [P, NP_], f16),
        ("xr0T", [P, NC_NODES], f16),
        ("xlidxlo", [P, NIL * 8], i16),
        ("xlidxhi", [P, NIH * 8], i16),
        ("oh", [P, NCH, P], f8),
        ("ohT", [P, NCH, P], f8),
        ("ident", [P, P], f8),
        ("attr0", [P, P], f16), ("attr1", [P, P], f16), ("attr2", [P, P], f16),
        ("wlr0", [P, 256], f16), ("wlr1", [P, 256], f16), ("wlr2", [P, 256], f16),
        ("pooloh", [P, NTC, G_GRAPHS], f16),
    ]:
        inp[name] = nc.dram_tensor(name, shape, dt, kind="ExternalInput")

    pooled = nc.dram_tensor("pooled", [G_GRAPHS, G_GRAPHS], f32,
                            kind="ExternalOutput")
    dbg = {}
    if os.environ.get("GAT_DEBUG"):
        dbg["xl0"] = nc.dram_tensor("dbg_xl0", [NP_, P], f16,
                                    kind="ExternalOutput")
        dbg["xn0"] = nc.dram_tensor("dbg_xn0", [NC_NODES, P], f16,
                                    kind="ExternalOutput")
        dbg["xl1"] = nc.dram_tensor("dbg_xl1", [NP_, P], f16,
                                    kind="ExternalOutput")
        dbg["st0"] = nc.dram_tensor("dbg_st0", [P, 64, P], f16,
                                    kind="ExternalOutput")
        dbg["L0"] = nc.dram_tensor("dbg_L0", [P, 64, P], f16,
                                   kind="ExternalOutput")
        dbg["w0"] = nc.dram_tensor("dbg_w0", [P, 64, P + 4], f16,
                                   kind="ExternalOutput")

    # xl_full[l]: per-edge gather source, rows in (chunk, core, row) slot order
    xl_full = [
        nc.dram_tensor("xl_full0", [NP_, P], f16),
        nc.dram_tensor("xl_full1", [NP_, P], f16, addr_space="Shared"),
        nc.dram_tensor("xl_full2", [NP_, P], f16),
    ]
    # layer-2 ag payload is only 64 wide; gathered rows must still be 256B,
    # so ag lands compact and a local DMA expands into xl_full2's row pitch
    xl2c = nc.dram_tensor("xl_full2c", [NP_, W_L[2]], f16, addr_space="Shared")
    w_own2 = P if os.environ.get("GAT_L2FULL") else W_L[2]
    xl_own = [
        None,
        nc.dram_tensor("xl_own1", [NC_NODES, P], f16),
        nc.dram_tensor("xl_own2", [NC_NODES, w_own2], f16),
    ]
    xn_own = nc.dram_tensor("xn_own", [NC_NODES, P], f16)

    grp_list = _group_list()
    # per-group descriptors: chunk lists per stream, cumulative offsets
    ginfo = {}
    a_lo = a_hi = a_o = 0
    for gi, g in grp_list:
        lo_list = [tt for tt in range(gi, gi + g) for _ in range(chs_lo[tt])]
        hi_list = [tt for tt in range(gi, gi + g) for _ in range(chs_hi[tt])]
        ginfo[gi] = dict(lo=lo_list, hi=hi_list, io_lo=a_lo, io_hi=a_hi,
                         oo=a_o)
        a_lo += len(lo_list) * 8
        a_hi += len(hi_list) * 8
        a_o += len(lo_list) + len(hi_list)

    SA = 8  # tiles per phase-A strip (shares the [P,8,128] psu PSUM tag)

    with tile.TileContext(nc) as tc:
        with (
            tc.tile_pool(name="const", bufs=1) as cpool,
            tc.tile_pool(name="stage", bufs=1) as spool,
            tc.tile_pool(name="strip", bufs=2) as stpool,
            tc.tile_pool(name="edge", bufs=2) as epool,
            tc.tile_pool(name="small", bufs=2) as smpool,
            tc.tile_pool(name="psU", bufs=2, space="PSUM") as psU,
            tc.tile_pool(name="psS", bufs=2, space="PSUM") as psS,
            tc.tile_pool(name="psP", bufs=1, space="PSUM") as psP,
        ):
            ident_t = cpool.tile([P, P], f8)
            nc.sync.dma_start(out=ident_t[:], in_=inp["ident"][:])
            pool_t = cpool.tile([P, NTC, G_GRAPHS], f16)
            nc.sync.dma_start(out=pool_t[:], in_=inp["pooloh"][:])
            wlr_t, att_t = [], []
            for l in range(3):
                w_t_ = cpool.tile([P, 256], f16, tag=f"wlr{l}")
                nc.sync.dma_start(out=w_t_[:], in_=inp[f"wlr{l}"][:])
                wlr_t.append(w_t_)
                a_t_ = cpool.tile([P, P], f16, tag=f"att{l}")
                nc.sync.dma_start(out=a_t_[:], in_=inp[f"attr{l}"][:])
                att_t.append(a_t_)

            pool_psum = psP.tile([G_GRAPHS, G_GRAPHS], f32, space="PSUM")

            # persistent per-layer state
            xr_sb = spool.tile([P, NTC, P], f16, tag="xr_sb")
            stg_xl = spool.tile([P, NTC, P], f16, tag="stg_xl")
            stg_xn = spool.tile([P, NTC, P], f16, tag="stg_xn")

            ncopy = [0]

            def psum_copy(dst, src):
                # alternate PSUM->SBUF copies between ACT and DVE
                if ncopy[0] % 2 == 0:
                    nc.scalar.copy(out=dst, in_=src)
                else:
                    nc.vector.tensor_copy(out=dst, in_=src)
                ncopy[0] += 1

            # ---- replicated phase A, layer 0: xl0 for ALL slots ----
            for s0 in range(0, NT, SA):
                w_ = min(SA, NT - s0)
                xs_t = stpool.tile([P, SA * P], f16, tag="xstrip")
                nc.sync.dma_start(out=xs_t[:, :w_ * P],
                                  in_=inp["x0T"][:, s0 * P:(s0 + w_) * P])
                ps = psU.tile([P, SA, P], f32, space="PSUM", tag="psu")
                for j in range(w_):
                    nc.tensor.matmul(
                        out=ps[:, j, :], lhsT=xs_t[:, j * P:(j + 1) * P],
                        rhs=wlr_t[0][:, :P], start=True, stop=True)
                stg = stpool.tile([P, SA, P], f16, tag="a0stg")
                psum_copy(stg[:, :w_, :], ps[:, :w_, :])
                nc.sync.dma_start(
                    out=xl_full[0][s0 * P:(s0 + w_) * P, :].rearrange(
                        "(t p) f -> p t f", p=P),
                    in_=stg[:, :w_, :])
            # layer-0 xr for own tiles
            for s0 in range(0, NTC, SA):
                w_ = min(SA, NTC - s0)
                xs_t = stpool.tile([P, SA * P], f16, tag="xstrip")
                nc.sync.dma_start(out=xs_t[:, :w_ * P],
                                  in_=inp["xr0T"][:, s0 * P:(s0 + w_) * P])
                ps = psU.tile([P, SA, P], f32, space="PSUM", tag="psu")
                for j in range(w_):
                    nc.tensor.matmul(
                        out=ps[:, j, :], lhsT=xs_t[:, j * P:(j + 1) * P],
                        rhs=wlr_t[0][:, 128:256], start=True, stop=True)
                psum_copy(xr_sb[:, s0:s0 + w_, :], ps[:, :w_, :])

            # ---- layers ----
            for l in range(3):
                Hh = H_L[l]
                W = W_L[l]
                CW = W // Hh
                att_l = att_t[l]

                for tt0, sz in CHUNKS:
                    for gi, g in _chunk_groups(tt0, sz):
                        info = ginfo[gi]
                        lo_list, hi_list = info["lo"], info["hi"]
                        nch_lo, nch_hi = len(lo_list), len(hi_list)
                        nch2 = nch_lo + nch_hi
                        col0 = info["oo"]

                        ilo = smpool.tile([P, GROUP * MCH * 8], i16, tag="ilo")
                        nc.sync.dma_start(
                            out=ilo[:, :nch_lo * 8],
                            in_=inp["xlidxlo"][
                                :, info["io_lo"]:info["io_lo"] + nch_lo * 8])
                        ihi = smpool.tile([P, GROUP * MCH * 8], i16, tag="ihi")
                        nc.sync.dma_start(
                            out=ihi[:, :nch_hi * 8],
                            in_=inp["xlidxhi"][
                                :, info["io_hi"]:info["io_hi"] + nch_hi * 8])
                        oh_t = epool.tile([P, 2 * GROUP * MCH, P], f8,
                                          tag="oh")
                        nc.sync.dma_start(
                            out=oh_t[:, :nch2, :],
                            in_=inp["oh"][:, col0:col0 + nch2, :])
                        ohT_t = epool.tile([P, 2 * GROUP * MCH, P], f8,
                                           tag="ohT")
                        for o0 in range(0, nch2, BATCH):
                            on = min(BATCH, nch2 - o0)
                            nc.sync.dma_start(
                                out=ohT_t[:, o0:o0 + on, :],
                                in_=inp["ohT"][:, col0 + o0:col0 + o0 + on, :])

                        # gather calls (<= GMAX idxs each) per stream;
                        # GMAX must stay <= half the SWDGE ring (2048 descs)
                        GMAX = 1024
                        def gathers(st, in_ap, idx_t, nch_s):
                            k = 0
                            while k < nch_s * P:
                                n = min(GMAX, nch_s * P - k)
                                nc.gpsimd.dma_gather(
                                    out_ap=st[:, k // P:(k + n) // P, :],
                                    in_ap=in_ap,
                                    idxs_ap=idx_t[:, k // 16:(k + n) // 16],
                                    num_idxs=n, num_idxs_reg=n, elem_size=P)
                                k += n
                        st_lo = epool.tile([P, GROUP * MCH, P], f16,
                                           tag="xlglo")
                        gathers(st_lo, xl_full[l][0:HALF, :], ilo, nch_lo)
                        st_hi = epool.tile([P, GROUP * MCH, P], f16,
                                           tag="xlghi")
                        gathers(st_hi, xl_full[l][HALF:NP_, :], ihi, nch_hi)

                        # u = xr[tloc] + xl_src  (PSUM), leaky-relu -> L
                        L_t = epool.tile([P, 2 * GROUP * MCH, P], f16, tag="L")
                        for c0, xt, clist in ((0, st_lo, lo_list),
                                              (nch_lo, st_hi, hi_list)):
                            for b0 in range(0, len(clist), BATCH):
                                nb = min(BATCH, len(clist) - b0)
                                psu = psU.tile([P, BATCH, P], f32,
                                               space="PSUM", tag="psu")
                                for k in range(nb):
                                    cc = b0 + k
                                    nc.tensor.matmul(
                                        out=psu[:, k, :W],
                                        lhsT=ohT_t[:, c0 + cc, :],
                                        rhs=xr_sb[:, clist[cc], :W],
                                        start=True, stop=False)
                                    nc.tensor.matmul(
                                        out=psu[:, k, :W], lhsT=ident_t[:],
                                        rhs=xt[:, cc, :W],
                                        start=False, stop=True)
                                nc.scalar.activation(
                                    out=L_t[:, c0 + b0:c0 + b0 + nb, :W],
                                    in_=psu[:, :nb, :W],
                                    func=mybir.ActivationFunctionType.Prelu,
                                    alpha=NEG)

                        # scores: L *= att ; tree-reduce over cw
                        nc.vector.tensor_tensor(
                            out=L_t[:, :nch2, :W], in0=L_t[:, :nch2, :W],
                            in1=att_l[:, :W].unsqueeze(1).broadcast_to(
                                [P, nch2, W]),
                            op=mybir.AluOpType.mult)
                        w_t = epool.tile([P, 2 * GROUP * MCH, P + 4], f16,
                                         tag="w")
                        Lv = L_t[:, :nch2, :W].rearrange(
                            "p c (w h) -> p c w h", h=Hh)
                        # tree halves in place into L_t's low columns (L is
                        # dead after the att-mult; keeps w_t alias-free)
                        tv = L_t[:, :nch2, :W // 2].rearrange(
                            "p c (w h) -> p c w h", h=Hh)
                        half = CW // 2
                        nc.vector.tensor_tensor(
                            out=tv[:, :, :half, :], in0=Lv[:, :, :half, :],
                            in1=Lv[:, :, half:, :], op=mybir.AluOpType.add)
                        while half > 1:
                            q = half // 2
                            nc.vector.tensor_tensor(
                                out=tv[:, :, :q, :], in0=tv[:, :, :q, :],
                                in1=tv[:, :, q:half, :], op=mybir.AluOpType.add)
                            half = q
                        # w values and alpha
                        nc.scalar.activation(
                            out=w_t[:, :nch2, W:W + Hh], in_=tv[:, :nch2, 0, :],
                            func=mybir.ActivationFunctionType.Exp)
                        a_b = w_t[:, :nch2, W:W + Hh].unsqueeze(2).broadcast_to(
                            [P, nch2, CW, Hh])
                        for c0, xt, clist in ((0, st_lo, lo_list),
                                              (nch_lo, st_hi, hi_list)):
                            ns = len(clist)
                            nc.vector.tensor_tensor(
                                out=w_t[:, c0:c0 + ns, :W].rearrange(
                                    "p c (w h) -> p c w h", h=Hh),
                                in0=xt[:, :ns, :W].rearrange(
                                    "p c (w h) -> p c w h", h=Hh),
                                in1=a_b[:, c0:c0 + ns],
                                op=mybir.AluOpType.mult)

                        if dbg and l == 0 and gi == 0:
                            nc.sync.dma_start(out=dbg["st0"][:, :nch_lo, :],
                                              in_=st_lo[:, :nch_lo, :])
                            nc.sync.dma_start(out=dbg["L0"][:, :nch2, :],
                                              in_=L_t[:, :nch2, :])
                            nc.sync.dma_start(out=dbg["w0"][:, :nch2, :],
                                              in_=w_t[:, :nch2, :])

                        # scatter per tile
                        for tt_ in range(g):
                            t = gi + tt_
                            cids = ([c0 for c0, tt in enumerate(lo_list)
                                     if tt == t]
                                    + [nch_lo + c0
                                       for c0, tt in enumerate(hi_list)
                                       if tt == t])
                            ps = psS.tile([P, P + 4], f32, space="PSUM",
                                          tag="pss")
                            for cix, cid in enumerate(cids):
                                nc.tensor.matmul(
                                    out=ps[:, :W + Hh],
                                    lhsT=oh_t[:, cid, :],
                                    rhs=w_t[:, cid, :W + Hh],
                                    start=(cix == 0),
                                    stop=(cix == len(cids) - 1))
                            rec = smpool.tile([P, 4], f32, tag="rec")
                            nc.vector.reciprocal(out=rec[:, :Hh],
                                                 in_=ps[:, W:W + Hh])
                            t1 = smpool.tile([P, P], f16, tag="t1")
                            nc.vector.tensor_tensor(
                                out=t1[:, :W].rearrange(
                                    "p (w h) -> p w h", h=Hh),
                                in0=ps[:, :W].rearrange(
                                    "p (w h) -> p w h", h=Hh),
                                in1=rec[:, :Hh].unsqueeze(1).broadcast_to(
                                    [P, CW, Hh]),
                                op=mybir.AluOpType.mult)
                            if l < 2:
                                nc.scalar.activation(
                                    out=stg_xn[:, t, :], in_=t1[:],
                                    func=mybir.ActivationFunctionType.Prelu,
                                    alpha=NEG)
                            else:
                                xnm = smpool.tile([P, G_GRAPHS], f16,
                                                  tag="xnm2")
                                nc.scalar.activation(
                                    out=xnm[:], in_=t1[:, :G_GRAPHS],
                                    func=mybir.ActivationFunctionType.Prelu,
                                    alpha=NEG)
                                nc.tensor.matmul(
                                    out=pool_psum[:],
                                    lhsT=pool_t[:, t, :], rhs=xnm[:],
                                    start=(t == 0), stop=(t == NTC - 1))

                    # ---- interleaved phase A(l+1) + chunk AllGather ----
                    if l < 2:
                        Wn = W_L[l + 1]
                        nc.sync.dma_start(
                            out=xn_own[tt0 * P:(tt0 + sz) * P, :].rearrange(
                                "(t p) f -> p t f", p=P),
                            in_=stg_xn[:, tt0:tt0 + sz, :])
                        for j0 in range(tt0, tt0 + sz, 2):
                            w_ = min(2, tt0 + sz - j0)
                            xs_t = stpool.tile([P, 2 * P], f16, tag="xstrip2")
                            nc.sync.dma_start_transpose(
                                out=xs_t[:, :w_ * P],
                                in_=xn_own[j0 * P:(j0 + w_) * P, :])
                            ps = psU.tile([P, SA, P], f32, space="PSUM",
                                          tag="psu")
                            for j in range(w_):
                                nc.tensor.matmul(
                                    out=ps[:, j, :Wn],
                                    lhsT=xs_t[:, j * P:(j + 1) * P],
                                    rhs=wlr_t[l + 1][:, :Wn],
                                    start=True, stop=True)
                                nc.tensor.matmul(
                                    out=ps[:, 4 + j, :Wn],
                                    lhsT=xs_t[:, j * P:(j + 1) * P],
                                    rhs=wlr_t[l + 1][:, 128:128 + Wn],
                                    start=True, stop=True)
                            psum_copy(stg_xl[:, j0:j0 + w_, :Wn],
                                      ps[:, :w_, :Wn])
                            psum_copy(xr_sb[:, j0:j0 + w_, :Wn],
                                      ps[:, 4:4 + w_, :Wn])
                        Ws = xl_own[l + 1].shape[1]
                        nc.sync.dma_start(
                            out=xl_own[l + 1][tt0 * P:(tt0 + sz) * P, :]
                            .rearrange("(t p) f -> p t f", p=P),
                            in_=stg_xl[:, tt0:tt0 + sz, :Ws])
                        s0 = CORES * tt0 * P
                        s1 = CORES * (tt0 + sz) * P
                        if Wn == P or os.environ.get("GAT_L2FULL"):
                            nc.gpsimd.collective_compute(
                                "AllGather", mybir.AluOpType.bypass,
                                replica_groups=[list(range(CORES))],
                                ins=[xl_own[l + 1][tt0 * P:(tt0 + sz) * P, :]],
                                outs=[xl_full[l + 1][s0:s1, :]])
                        else:
                            nc.gpsimd.collective_compute(
                                "AllGather", mybir.AluOpType.bypass,
                                replica_groups=[list(range(CORES))],
                                ins=[xl_own[l + 1][tt0 * P:(tt0 + sz) * P, :]],
                                outs=[xl2c[s0:s1, :]])
                            nc.sync.dma_start(
                                out=xl_full[l + 1][s0:s1, :Wn],
                                in_=xl2c[s0:s1, :])

                if dbg and l == 0:
                    nc.sync.dma_start(out=dbg["xl0"][:], in_=xl_full[0][:])
                    nc.sync.dma_start(out=dbg["xn0"][:], in_=xn_own[:])
                if dbg and l == 1:
                    nc.sync.dma_start(out=dbg["xl1"][:], in_=xl_full[1][:])

            pool_sb = smpool.tile([G_GRAPHS, G_GRAPHS], f32, tag="poolsb")
            nc.vector.tensor_copy(out=pool_sb[:], in_=pool_psum[:])
            nc.sync.dma_start(out=pooled[:], in_=pool_sb[:])

    nc.finalize()
    return nc


def kernel(**inputs):
    x = np.asarray(inputs["x"])
    edge_index = np.asarray(inputs["edge_index"])
    batch = np.asarray(inputs["batch"])
    params = []
    for l in range(3):
        params.append((np.asarray(inputs[f"Wl{l}"]),
                       np.asarray(inputs[f"Wr{l}"]),
                       np.asarray(inputs[f"att{l}"])))
        b = np.asarray(inputs[f"b{l}"])
        assert np.all(b == 0), "nonzero bias not supported"

    meta, in_maps = _preprocess(x, edge_index, batch, params)

    key = ("nc", meta["chs_lo"], meta["chs_hi"])
    if key not in _CACHE:
        _CACHE[key] = _build(meta)
    nc = _CACHE[key]

    try:
        res = run_bass_kernel_spmd(
            nc, in_maps, core_ids=list(range(CORES)),
            trace=bool(os.environ.get("GAT_TRACE")))
    except ModuleNotFoundError:
        res = run_bass_kernel_spmd(nc, in_maps, core_ids=list(range(CORES)))
    kernel._last_result = res

    pooled = np.zeros((G_GRAPHS, G_GRAPHS), np.float64)
    for c in range(CORES):
        pooled += res.results[c]["pooled"].astype(np.float64)
    cnt = np.bincount(batch, minlength=G_GRAPHS).astype(np.float64)
    out = pooled / np.maximum(cnt, 1.0)[:, None]
    return out.astype(np.float32)


# revision 50
# speedup vs baseline: 1.0360x; 1.0360x over previous
"""GATv2 (3-layer, heads=4/4/1) full-graph kernel for 8 Trainium2 NeuronCores.

Contract: kernel(**inputs) takes the FULL unsharded inputs (as produced by
setup_inputs()) and returns the FULL [64, 64] float32 output.

v4 design (vs. v3 baseline at 1.90 ms):
- Layer-0 phase A is REPLICATED: every core computes xl0 for all 50176 nodes
  straight from the (fully available) input x and writes it to local DRAM.
  The 251 us layer-0 feature AllGather is gone entirely.
- Layers 1-2 exchange xl via CHUNKED AllGathers overlapped with compute:
  phase A(l+1) for a chunk of own tiles runs as soon as phase B(l) finishes
  those tiles, and the chunk's AllGather fires immediately, running on the
  collective cores while phase B(l) continues on later chunks.  xl_full rows
  are laid out (chunk, core, row)-major so every AllGather lands in a
  contiguous slice with IR identical on all cores.
- Gathers issue one dma_gather per (group, stream) (up to 3840 indices per
  call, 8192-descriptor SWDGE ring) to amortize the ~1 us fixed SWDGE cost.
- Pad targets (node ids >= N) get a fake self-loop so every target has a
  nonzero softmax denominator; per-tile normalization is then a single DVE
  divide (no max/reciprocal dance, no NaNs reaching the pool matmul).
- Per-edge xr is expanded on the PE from SBUF-resident xr tiles via host-built
  fp8 one-hot matrices (oh: [lane,tgt], ohT: [tgt,lane]); gathered xl rows are
  accumulated into the same PSUM via an identity matmul; leaky-relu applied
  straight from PSUM on ACT; scores via DVE mult + halving-tree; softmax
  without max-shift (scores empirically in [-8, 7]); scatter-sum + denominators
  via fp8 one-hot matmul into PSUM; global-mean-pool partials via PE, summed
  and divided on the host.
"""
import os
import numpy as np
import ml_dtypes

import concourse.bacc as bacc
import concourse.mybir as mybir
import concourse.tile as tile
from concourse._compat import get_trn_type
from concourse.bass_utils import run_bass_kernel_spmd

f16 = mybir.dt.float16
f32 = mybir.dt.float32
f8 = mybir.dt.float8e4
i16 = mybir.dt.int16
f8np = ml_dtypes.float8_e4m3

P = 128
N = 50000
E = 800000
NP_ = 50176            # padded nodes = 392 * 128
NT = NP_ // P          # 392 global tiles
CORES = 8
NTC = NT // CORES      # 49 tiles per core
NC_NODES = NTC * P     # 6272 nodes per core
HALF = NP_ // 2        # 25088 rows per shared half
G_GRAPHS = 64
NEG = 0.2
GROUP = 3              # tiles per gather/compute group
BATCH = 8              # chunks per PSUM u-batch
H_L = [4, 4, 1]
W_L = [128, 128, 64]   # xl/value width per layer
CHUNKS = [(0, 13), (13, 12), (25, 12), (37, 12)]  # (tt0, sz) ag-pipeline chunks

_CACHE = {}


def _pack_idx_image(seq):
    """int16 index sequence -> gather SBUF image [128, len/16]."""
    n = len(seq)
    assert n % 16 == 0
    img = np.asarray(seq, np.int16).reshape(n // 16, 16).T
    return np.tile(img, (8, 1))


def _chunk_groups(tt0, sz):
    out = []
    gi = tt0
    while gi < tt0 + sz:
        out.append((gi, min(GROUP, tt0 + sz - gi)))
        gi += GROUP
    return out


def _group_list():
    out = []
    for tt0, sz in CHUNKS:
        out.extend(_chunk_groups(tt0, sz))
    return out


def _tile_slot():
    """global tile t -> slot in the (chunk, core, row) xl_full layout."""
    slot = np.empty(NT, np.int64)
    for c in range(CORES):
        for tt0, sz in CHUNKS:
            for i in range(sz):
                slot[NTC * c + tt0 + i] = CORES * tt0 + c * sz + i
    return slot


def _pack_perm(h, c):
    """column permutation: packed[cw*h_n + hh] = natural[hh*c + cw]."""
    perm = np.empty(h * c, np.int64)
    for cw in range(c):
        for hh in range(h):
            perm[cw * h + hh] = hh * c + cw
    return perm


def _balance_perm(edge_index):
    """Relabel real nodes so every 128-node tile has near-equal in-degree.
    Returns perm[orig] -> new position (pads N..NP_ stay in place)."""
    import heapq
    deg = np.bincount(edge_index[1].astype(np.int64), minlength=N) + 1
    order = np.argsort(-deg, kind="stable")
    nfull = N // P                      # 390 full tiles
    caps = [P] * nfull + [N - nfull * P]  # tile 390 gets the remainder
    heap = [(0, b) for b in range(len(caps))]
    heapq.heapify(heap)
    fill = [0] * len(caps)
    perm = np.empty(N, np.int64)
    for v in order:
        while True:
            s, b = heapq.heappop(heap)
            if fill[b] < caps[b]:
                break
        perm[v] = b * P + fill[b]
        fill[b] += 1
        if fill[b] < caps[b]:
            heapq.heappush(heap, (s + int(deg[v]), b))
    return perm


def _preprocess(x, edge_index, batch, params):
    nperm = _balance_perm(edge_index)
    loops = np.arange(N, dtype=np.int64)
    pads = np.arange(N, NP_, dtype=np.int64)   # fake self-loops on pad targets
    src = np.concatenate([nperm[edge_index[0].astype(np.int64)], nperm[loops],
                          pads])
    tgt = np.concatenate([nperm[edge_index[1].astype(np.int64)], nperm[loops],
                          pads])
    order = np.argsort(tgt, kind="stable")
    srcs, tgts = src[order], tgt[order]

    slot = _tile_slot()
    src_row = slot[srcs // P] * P + srcs % P   # permuted xl_full row per edge
    islo = src_row < HALF

    bounds = np.searchsorted(tgts, np.arange(0, NP_ + 1, P))
    nlo = np.empty(NT, np.int64)
    nhi = np.empty(NT, np.int64)
    for t in range(NT):
        s, e = bounds[t], bounds[t + 1]
        nlo[t] = int(islo[s:e].sum())
        nhi[t] = (e - s) - nlo[t]
    # per-core-tile-slot chunk counts (max over cores, static across SPMD IR)
    chs_lo = tuple(
        int(max(1, -(-nlo[tt::NTC].max() // P))) for tt in range(NTC))
    chs_hi = tuple(
        int(max(1, -(-nhi[tt::NTC].max() // P))) for tt in range(NTC))

    # per-layer packed weights / attention
    wlrs, att_reps = [], []
    prev_perm = None  # input-feature permutation (packing of previous layer)
    for li, (Wl, Wr, att) in enumerate(params):
        h, c = att.shape
        hc = h * c
        Wl = np.asarray(Wl, np.float32)
        Wr = np.asarray(Wr, np.float32)
        if prev_perm is not None:
            Wl = Wl[prev_perm]
            Wr = Wr[prev_perm]
        if li < 2:
            perm = _pack_perm(h, c)
            Wl = Wl[:, perm]
            Wr = Wr[:, perm]
            att_flat = np.asarray(att, np.float32).reshape(-1)[perm]
            prev_perm = perm
        else:
            att_flat = np.asarray(att, np.float32).reshape(-1)
            prev_perm = None
        wlr = np.zeros((P, 256), np.float16)
        wlr[: Wl.shape[0], :hc] = Wl.astype(np.float16)
        wlr[: Wr.shape[0], 128 : 128 + hc] = Wr.astype(np.float16)
        wlrs.append(wlr)
        af = np.zeros(P, np.float16)
        af[:hc] = att_flat.astype(np.float16)
        att_reps.append(np.tile(af[None, :], (P, 1)))

    ident = np.eye(P).astype(f8np)

    x_pad = np.zeros((NP_, P), np.float32)
    x_pad[nperm] = np.asarray(x, np.float32)   # rows at balanced positions
    # x columns in slot-major (permuted) order, shared by all cores
    slot_tile = np.empty(NT, np.int64)
    slot_tile[slot] = np.arange(NT)
    xTp = np.empty((P, NP_), np.float16)
    for s in range(NT):
        t = slot_tile[s]
        xTp[:, s * P:(s + 1) * P] = x_pad[t * P:(t + 1) * P].astype(np.float16).T

    grp_list = _group_list()

    # graph id per NEW position (pads -> 0, masked out by valid)
    batch_perm = np.zeros(NP_, np.int64)
    batch_perm[nperm] = np.asarray(batch, np.int64)
    valid_perm = np.zeros(NP_, bool)
    valid_perm[nperm] = True

    in_maps = []
    for c in range(CORES):
        t0 = c * NTC
        base = t0 * P
        # per-(tile, stream) slot tables, padded to chs_*[tt]*128
        xlo = [np.zeros(chs_lo[tt] * P, np.int64) for tt in range(NTC)]
        xhi = [np.zeros(chs_hi[tt] * P, np.int64) for tt in range(NTC)]
        tl_lo = [np.full(chs_lo[tt] * P, -1, np.int64) for tt in range(NTC)]
        tl_hi = [np.full(chs_hi[tt] * P, -1, np.int64) for tt in range(NTC)]
        for tt in range(NTC):
            t = t0 + tt
            s, e = bounds[t], bounds[t + 1]
            sl = tgts[s:e] - t * P
            sp = src_row[s:e]
            lo_mask = islo[s:e]
            k = int(lo_mask.sum()); k2 = (e - s) - k
            xlo[tt][:k] = sp[lo_mask]
            tl_lo[tt][:k] = sl[lo_mask]
            xhi[tt][:k2] = sp[~lo_mask] - HALF
            tl_hi[tt][:k2] = sl[~lo_mask]

        # group-stream-major chunk columns
        lo_imgs, hi_imgs = [], []
        oh_cols, ohT_cols = [], []
        for gi, g in grp_list:
            lo_seq = np.concatenate(xlo[gi:gi + g])
            hi_seq = np.concatenate(xhi[gi:gi + g])
            lo_imgs.append(_pack_idx_image(lo_seq))
            hi_imgs.append(_pack_idx_image(hi_seq))
            tl_seq = np.concatenate(tl_lo[gi:gi + g] + tl_hi[gi:gi + g])
            nch2 = len(tl_seq) // P
            tl_mat = tl_seq.reshape(nch2, P)          # [chunk, lane] -> tloc
            oh = np.zeros((P, nch2, P), f8np)         # [lane, chunk, tgt]
            ohT = np.zeros((P, nch2, P), f8np)        # [tgt, chunk, lane]
            ch_i, ln_i = np.nonzero(tl_mat >= 0)
            tl_v = tl_mat[ch_i, ln_i]
            oh[ln_i, ch_i, tl_v] = 1.0
            ohT[tl_v, ch_i, ln_i] = 1.0
            oh_cols.append(oh)
            ohT_cols.append(ohT)

        # pooling one-hot [128, NTC, 64]
        pool = np.zeros((P, NTC, G_GRAPHS), np.float16)
        for tt in range(NTC):
            gn = base + tt * P + np.arange(P)
            valid = valid_perm[gn]
            pool[valid, tt, batch_perm[gn[valid]]] = 1.0

        # own-shard x columns (natural tt order) for the layer-0 xr pass
        xr0T = np.ascontiguousarray(
            x_pad[base:base + NC_NODES].astype(np.float16).T)

        in_maps.append({
            "x0T": xTp,
            "xr0T": xr0T,
            "xlidxlo": np.concatenate(lo_imgs, axis=1),
            "xlidxhi": np.concatenate(hi_imgs, axis=1),
            "oh": np.concatenate(oh_cols, axis=1),
            "ohT": np.concatenate(ohT_cols, axis=1),
            "ident": ident,
            "attr0": att_reps[0], "attr1": att_reps[1], "attr2": att_reps[2],
            "wlr0": wlrs[0], "wlr1": wlrs[1], "wlr2": wlrs[2],
            "pooloh": pool,
        })

    return dict(chs_lo=chs_lo, chs_hi=chs_hi), in_maps


def _build(meta):
    chs_lo, chs_hi = meta["chs_lo"], meta["chs_hi"]
    NIL = sum(chs_lo)   # lo chunks per core
    NIH = sum(chs_hi)
    NCH = NIL + NIH     # total chunk columns per core
    MCH = max(max(chs_lo), max(chs_hi))
    nc = bacc.Bacc(
        get_trn_type() or "TRN2",
        target_bir_lowering=False,
        debug=False,
        num_devices=CORES,
        dynamic_dma_scratch_size=32768,   # 2048-descriptor SWDGE ring
    )
    inp = {}
    for name, shape, dt in [
        ("x0T", [P, NP_], f16),
        ("xr0T", [P, NC_NODES], f16),
        ("xlidxlo", [P, NIL * 8], i16),
        ("xlidxhi", [P, NIH * 8], i16),
        ("oh", [P, NCH, P], f8),
        ("ohT", [P, NCH, P], f8),
        ("ident", [P, P], f8),
        ("attr0", [P, P], f16), ("attr1", [P, P], f16), ("attr2", [P, P], f16),
        ("wlr0", [P, 256], f16), ("wlr1", [P, 256], f16), ("wlr2", [P, 256], f16),
        ("pooloh", [P, NTC, G_GRAPHS], f16),
    ]:
        inp[name] = nc.dram_tensor(name, shape, dt, kind="ExternalInput")

    pooled = nc.dram_tensor("pooled", [G_GRAPHS, G_GRAPHS], f32,
                            kind="ExternalOutput")
    dbg = {}
    if os.environ.get("GAT_DEBUG"):
        dbg["xl0"] = nc.dram_tensor("dbg_xl0", [NP_, P], f16,
                                    kind="ExternalOutput")
        dbg["xn0"] = nc.dram_tensor("dbg_xn0", [NC_NODES, P], f16,
                                    kind="ExternalOutput")
        dbg["xl1"] = nc.dram_tensor("dbg_xl1", [NP_, P], f16,
                                    kind="ExternalOutput")
        dbg["st0"] = nc.dram_tensor("dbg_st0", [P, 64, P], f16,
                                    kind="ExternalOutput")
        dbg["L0"] = nc.dram_tensor("dbg_L0", [P, 64, P], f16,
                                   kind="ExternalOutput")
        dbg["w0"] = nc.dram_tensor("dbg_w0", [P, 64, P + 4], f16,
                                   kind="ExternalOutput")

    # xl_full[l]: per-edge gather source, rows in (chunk, core, row) slot order
    xl_full = [
        nc.dram_tensor("xl_full0", [NP_, P], f16),
        nc.dram_tensor("xl_full1", [NP_, P], f16, addr_space="Shared"),
        nc.dram_tensor("xl_full2", [NP_, P], f16),
    ]
    # layer-2 ag payload is only 64 wide; gathered rows must still be 256B,
    # so ag lands compact and a local DMA expands into xl_full2's row pitch
    xl2c = nc.dram_tensor("xl_full2c", [NP_, W_L[2]], f16, addr_space="Shared")
    w_own2 = P if os.environ.get("GAT_L2FULL") else W_L[2]
    xl_own = [
        None,
        nc.dram_tensor("xl_own1", [NC_NODES, P], f16),
        nc.dram_tensor("xl_own2", [NC_NODES, w_own2], f16),
    ]
    xn_own = nc.dram_tensor("xn_own", [NC_NODES, P], f16)

    grp_list = _group_list()
    # per-group descriptors: chunk lists per stream, cumulative offsets
    ginfo = {}
    a_lo = a_hi = a_o = 0
    for gi, g in grp_list:
        lo_list = [tt for tt in range(gi, gi + g) for _ in range(chs_lo[tt])]
        hi_list = [tt for tt in range(gi, gi + g) for _ in range(chs_hi[tt])]
        ginfo[gi] = dict(lo=lo_list, hi=hi_list, io_lo=a_lo, io_hi=a_hi,
                         oo=a_o)
        a_lo += len(lo_list) * 8
        a_hi += len(hi_list) * 8
        a_o += len(lo_list) + len(hi_list)

    SA = 8  # tiles per phase-A strip (shares the [P,8,128] psu PSUM tag)

    with tile.TileContext(nc) as tc:
        with (
            tc.tile_pool(name="const", bufs=1) as cpool,
            tc.tile_pool(name="stage", bufs=1) as spool,
            tc.tile_pool(name="strip", bufs=2) as stpool,
            tc.tile_pool(name="edge", bufs=2) as epool,
            tc.tile_pool(name="small", bufs=2) as smpool,
            tc.tile_pool(name="psU", bufs=2, space="PSUM") as psU,
            tc.tile_pool(name="psS", bufs=2, space="PSUM") as psS,
            tc.tile_pool(name="psP", bufs=1, space="PSUM") as psP,
        ):
            ident_t = cpool.tile([P, P], f8)
            nc.sync.dma_start(out=ident_t[:], in_=inp["ident"][:])
            pool_t = cpool.tile([P, NTC, G_GRAPHS], f16)
            nc.sync.dma_start(out=pool_t[:], in_=inp["pooloh"][:])
            wlr_t, att_t = [], []
            for l in range(3):
                w_t_ = cpool.tile([P, 256], f16, tag=f"wlr{l}")
                nc.sync.dma_start(out=w_t_[:], in_=inp[f"wlr{l}"][:])
                wlr_t.append(w_t_)
                a_t_ = cpool.tile([P, P], f16, tag=f"att{l}")
                nc.sync.dma_start(out=a_t_[:], in_=inp[f"attr{l}"][:])
                att_t.append(a_t_)

            pool_psum = psP.tile([G_GRAPHS, G_GRAPHS], f32, space="PSUM")

            # persistent per-layer state
            xr_sb = spool.tile([P, NTC, P], f16, tag="xr_sb")
            stg_xl = spool.tile([P, NTC, P], f16, tag="stg_xl")
            stg_xn = spool.tile([P, NTC, P], f16, tag="stg_xn")

            ncopy = [0]

            def psum_copy(dst, src):
                # alternate PSUM->SBUF copies between ACT and DVE
                if ncopy[0] % 2 == 0:
                    nc.scalar.copy(out=dst, in_=src)
                else:
                    nc.vector.tensor_copy(out=dst, in_=src)
                ncopy[0] += 1

            # ---- replicated phase A, layer 0: xl0 for ALL slots ----
            for s0 in range(0, NT, SA):
                w_ = min(SA, NT - s0)
                xs_t = stpool.tile([P, SA * P], f16, tag="xstrip")
                nc.sync.dma_start(out=xs_t[:, :w_ * P],
                                  in_=inp["x0T"][:, s0 * P:(s0 + w_) * P])
                ps = psU.tile([P, SA, P], f32, space="PSUM", tag="psu")
                for j in range(w_):
                    nc.tensor.matmul(
                        out=ps[:, j, :], lhsT=xs_t[:, j * P:(j + 1) * P],
                        rhs=wlr_t[0][:, :P], start=True, stop=True)
                stg = stpool.tile([P, SA, P], f16, tag="a0stg")
                psum_copy(stg[:, :w_, :], ps[:, :w_, :])
                nc.sync.dma_start(
                    out=xl_full[0][s0 * P:(s0 + w_) * P, :].rearrange(
                        "(t p) f -> p t f", p=P),
                    in_=stg[:, :w_, :])
            # layer-0 xr for own tiles
            for s0 in range(0, NTC, SA):
                w_ = min(SA, NTC - s0)
                xs_t = stpool.tile([P, SA * P], f16, tag="xstrip")
                nc.sync.dma_start(out=xs_t[:, :w_ * P],
                                  in_=inp["xr0T"][:, s0 * P:(s0 + w_) * P])
                ps = psU.tile([P, SA, P], f32, space="PSUM", tag="psu")
                for j in range(w_):
                    nc.tensor.matmul(
                        out=ps[:, j, :], lhsT=xs_t[:, j * P:(j + 1) * P],
                        rhs=wlr_t[0][:, 128:256], start=True, stop=True)
                psum_copy(xr_sb[:, s0:s0 + w_, :], ps[:, :w_, :])

            # ---- layers ----
            for l in range(3):
                Hh = H_L[l]
                W = W_L[l]
                CW = W // Hh
                att_l = att_t[l]

                for tt0, sz in CHUNKS:
                    for gi, g in _chunk_groups(tt0, sz):
                        info = ginfo[gi]
                        lo_list, hi_list = info["lo"], info["hi"]
                        nch_lo, nch_hi = len(lo_list), len(hi_list)
                        nch2 = nch_lo + nch_hi
                        col0 = info["oo"]

                        ilo = smpool.tile([P, GROUP * MCH * 8], i16, tag="ilo")
                        nc.sync.dma_start(
                            out=ilo[:, :nch_lo * 8],
                            in_=inp["xlidxlo"][
                                :, info["io_lo"]:info["io_lo"] + nch_lo * 8])
                        ihi = smpool.tile([P, GROUP * MCH * 8], i16, tag="ihi")
                        nc.sync.dma_start(
                            out=ihi[:, :nch_hi * 8],
                            in_=inp["xlidxhi"][
                                :, info["io_hi"]:info["io_hi"] + nch_hi * 8])
                        oh_t = epool.tile([P, 2 * GROUP * MCH, P], f8,
                                          tag="oh")
                        nc.sync.dma_start(
                            out=oh_t[:, :nch2, :],
                            in_=inp["oh"][:, col0:col0 + nch2, :])
                        ohT_t = epool.tile([P, 2 * GROUP * MCH, P], f8,
                                           tag="ohT")
                        for o0 in range(0, nch2, BATCH):
                            on = min(BATCH, nch2 - o0)
                            nc.sync.dma_start(
                                out=ohT_t[:, o0:o0 + on, :],
                                in_=inp["ohT"][:, col0 + o0:col0 + o0 + on, :])

                        # gather calls (<= GMAX idxs each) per stream;
                        # GMAX must stay <= half the SWDGE ring (2048 descs)
                        GMAX = 1024
                        def gathers(st, in_ap, idx_t, nch_s):
                            k = 0
                            while k < nch_s * P:
                                n = min(GMAX, nch_s * P - k)
                                nc.gpsimd.dma_gather(
                                    out_ap=st[:, k // P:(k + n) // P, :],
                                    in_ap=in_ap,
                                    idxs_ap=idx_t[:, k // 16:(k + n) // 16],
                                    num_idxs=n, num_idxs_reg=n, elem_size=P)
                                k += n
                        st_lo = epool.tile([P, GROUP * MCH, P], f16,
                                           tag="xlglo")
                        gathers(st_lo, xl_full[l][0:HALF, :], ilo, nch_lo)
                        st_hi = epool.tile([P, GROUP * MCH, P], f16,
                                           tag="xlghi")
                        gathers(st_hi, xl_full[l][HALF:NP_, :], ihi, nch_hi)

                        # u = xr[tloc] + xl_src  (PSUM), leaky-relu -> L
                        L_t = epool.tile([P, 2 * GROUP * MCH, P], f16, tag="L")
                        for c0, xt, clist in ((0, st_lo, lo_list),
                                              (nch_lo, st_hi, hi_list)):
                            for b0 in range(0, len(clist), BATCH):
                                nb = min(BATCH, len(clist) - b0)
                                psu = psU.tile([P, BATCH, P], f32,
                                               space="PSUM", tag="psu")
                                for k in range(nb):
                                    cc = b0 + k
                                    nc.tensor.matmul(
                                        out=psu[:, k, :W],
                                        lhsT=ohT_t[:, c0 + cc, :],
                                        rhs=xr_sb[:, clist[cc], :W],
                                        start=True, stop=False)
                                    nc.tensor.matmul(
                                        out=psu[:, k, :W], lhsT=ident_t[:],
                                        rhs=xt[:, cc, :W],
                                        start=False, stop=True)
                                nc.scalar.activation(
                                    out=L_t[:, c0 + b0:c0 + b0 + nb, :W],
                                    in_=psu[:, :nb, :W],
                                    func=mybir.ActivationFunctionType.Prelu,
                                    alpha=NEG)

                        # scores: L *= att ; tree-reduce over cw
                        nc.vector.tensor_tensor(
                            out=L_t[:, :nch2, :W], in0=L_t[:, :nch2, :W],
                            in1=att_l[:, :W].unsqueeze(1).broadcast_to(
                                [P, nch2, W]),
                            op=mybir.AluOpType.mult)
                        w_t = epool.tile([P, 2 * GROUP * MCH, P + 4], f16,
                                         tag="w")
                        Lv = L_t[:, :nch2, :W].rearrange(
                            "p c (w h) -> p c w h", h=Hh)
                        # tree halves in place into L_t's low columns (L is
                        # dead after the att-mult; keeps w_t alias-free)
                        tv = L_t[:, :nch2, :W // 2].rearrange(
                            "p c (w h) -> p c w h", h=Hh)
                        half = CW // 2
                        nc.vector.tensor_tensor(
                            out=tv[:, :, :half, :], in0=Lv[:, :, :half, :],
                            in1=Lv[:, :, half:, :], op=mybir.AluOpType.add)
                        while half > 1:
                            q = half // 2
                            nc.vector.tensor_tensor(
                                out=tv[:, :, :q, :], in0=tv[:, :, :q, :],
                                in1=tv[:, :, q:half, :], op=mybir.AluOpType.add)
                            half = q
                        # w values and alpha
                        nc.scalar.activation(
                            out=w_t[:, :nch2, W:W + Hh], in_=tv[:, :nch2, 0, :],
                            func=mybir.ActivationFunctionType.Exp)
                        a_b = w_t[:, :nch2, W:W + Hh].unsqueeze(2).broadcast_to(
                            [P, nch2, CW, Hh])
                        for c0, xt, clist in ((0, st_lo, lo_list),
                                              (nch_lo, st_hi, hi_list)):
                            ns = len(clist)
                            nc.vector.tensor_tensor(
                                out=w_t[:, c0:c0 + ns, :W].rearrange(
                                    "p c (w h) -> p c w h", h=Hh),
                                in0=xt[:, :ns, :W].rearrange(
                                    "p c (w h) -> p c w h", h=Hh),
                                in1=a_b[:, c0:c0 + ns],
                                op=mybir.AluOpType.mult)

                        if dbg and l == 0 and gi == 0:
                            nc.sync.dma_start(out=dbg["st0"][:, :nch_lo, :],
                                              in_=st_lo[:, :nch_lo, :])
                            nc.sync.dma_start(out=dbg["L0"][:, :nch2, :],
                                              in_=L_t[:, :nch2, :])
                            nc.sync.dma_start(out=dbg["w0"][:, :nch2, :],
                                              in_=w_t[:, :nch2, :])

                        # scatter per tile
                        for tt_ in range(g):
                            t = gi + tt_
                            cids = ([c0 for c0, tt in enumerate(lo_list)
                                     if tt == t]
                                    + [nch_lo + c0
                                       for c0, tt in enumerate(hi_list)
                                       if tt == t])
                            ps = psS.tile([P, P + 4], f32, space="PSUM",
                                          tag="pss")
                            for cix, cid in enumerate(cids):
                                nc.tensor.matmul(
                                    out=ps[:, :W + Hh],
                                    lhsT=oh_t[:, cid, :],
                                    rhs=w_t[:, cid, :W + Hh],
                                    start=(cix == 0),
                                    stop=(cix == len(cids) - 1))
                            rec = smpool.tile([P, 4], f32, tag="rec")
                            nc.vector.reciprocal(out=rec[:, :Hh],
                                                 in_=ps[:, W:W + Hh])
                            t1 = smpool.tile([P, P], f16, tag="t1")
                            nc.vector.tensor_tensor(
                                out=t1[:, :W].rearrange(
                                    "p (w h) -> p w h", h=Hh),
                                in0=ps[:, :W].rearrange(
                                    "p (w h) -> p w h", h=Hh),
                                in1=rec[:, :Hh].unsqueeze(1).broadcast_to(
                                    [P, CW, Hh]),
                                op=mybir.AluOpType.mult)
                            if l < 2:
                                nc.scalar.activation(
                                    out=stg_xn[:, t, :], in_=t1[:],
                                    func=mybir.ActivationFunctionType.Prelu,
                                    alpha=NEG)
                            else:
                                xnm = smpool.tile([P, G_GRAPHS], f16,
                                                  tag="xnm2")
                                nc.scalar.activation(
                                    out=xnm[:], in_=t1[:, :G_GRAPHS],
                                    func=mybir.ActivationFunctionType.Prelu,
                                    alpha=NEG)
                                nc.tensor.matmul(
                                    out=pool_psum[:],
                                    lhsT=pool_t[:, t, :], rhs=xnm[:],
                                    start=(t == 0), stop=(t == NTC - 1))

                    # ---- interleaved phase A(l+1) + chunk AllGather ----
                    if l < 2:
                        Wn = W_L[l + 1]
                        nc.sync.dma_start(
                            out=xn_own[tt0 * P:(tt0 + sz) * P, :].rearrange(
                                "(t p) f -> p t f", p=P),
                            in_=stg_xn[:, tt0:tt0 + sz, :])
                        for j0 in range(tt0, tt0 + sz, 2):
                            w_ = min(2, tt0 + sz - j0)
                            xs_t = stpool.tile([P, 2 * P], f16, tag="xstrip2")
                            nc.sync.dma_start_transpose(
                                out=xs_t[:, :w_ * P],
                                in_=xn_own[j0 * P:(j0 + w_) * P, :])
                            ps = psU.tile([P, SA, P], f32, space="PSUM",
                                          tag="psu")
                            for j in range(w_):
                                nc.tensor.matmul(
                                    out=ps[:, j, :Wn],
                                    lhsT=xs_t[:, j * P:(j + 1) * P],
                                    rhs=wlr_t[l + 1][:, :Wn],
                                    start=True, stop=True)
                                nc.tensor.matmul(
                                    out=ps[:, 4 + j, :Wn],
                                    lhsT=xs_t[:, j * P:(j + 1) * P],
                                    rhs=wlr_t[l + 1][:, 128:128 + Wn],
                                    start=True, stop=True)
                            psum_copy(stg_xl[:, j0:j0 + w_, :Wn],
                                      ps[:, :w_, :Wn])
                            psum_copy(xr_sb[:, j0:j0 + w_, :Wn],
                                      ps[:, 4:4 + w_, :Wn])
                        Ws = xl_own[l + 1].shape[1]
                        nc.sync.dma_start(
                            out=xl_own[l + 1][tt0 * P:(tt0 + sz) * P, :]
                            .rearrange("(t p) f -> p t f", p=P),
                            in_=stg_xl[:, tt0:tt0 + sz, :Ws])
                        s0 = CORES * tt0 * P
                        s1 = CORES * (tt0 + sz) * P
                        if Wn == P or os.environ.get("GAT_L2FULL"):
                            nc.gpsimd.collective_compute(
                                "AllGather", mybir.AluOpType.bypass,
                                replica_groups=[list(range(CORES))],
                                ins=[xl_own[l + 1][tt0 * P:(tt0 + sz) * P, :]],
                                outs=[xl_full[l + 1][s0:s1, :]])
                        else:
                            nc.gpsimd.collective_compute(
                                "AllGather", mybir.AluOpType.bypass,
                                replica_groups=[list(range(CORES))],
                                ins=[xl_own[l + 1][tt0 * P:(tt0 + sz) * P, :]],
                                outs=[xl2c[s0:s1, :]])
                            nc.sync.dma_start(
                                out=xl_full[l + 1][s0:s1, :Wn],
                                in_=xl2c[s0:s1, :])

                if dbg and l == 0:
                    nc.sync.dma_start(out=dbg["xl0"][:], in_=xl_full[0][:])
                    nc.sync.dma_start(out=dbg["xn0"][:], in_=xn_own[:])
                if dbg and l == 1:
                    nc.sync.dma_start(out=dbg["xl1"][:], in_=xl_full[1][:])

            pool_sb = smpool.tile([G_GRAPHS, G_GRAPHS], f32, tag="poolsb")
            nc.vector.tensor_copy(out=pool_sb[:], in_=pool_psum[:])
            nc.sync.dma_start(out=pooled[:], in_=pool_sb[:])

    nc.finalize()
    return nc


def kernel(**inputs):
    x = np.asarray(inputs["x"])
    edge_index = np.asarray(inputs["edge_index"])
    batch = np.asarray(inputs["batch"])
    params = []
    for l in range(3):
        params.append((np.asarray(inputs[f"Wl{l}"]),
                       np.asarray(inputs[f"Wr{l}"]),
                       np.asarray(inputs[f"att{l}"])))
        b = np.asarray(inputs[f"b{l}"])
        assert np.all(b == 0), "nonzero bias not supported"

    meta, in_maps = _preprocess(x, edge_index, batch, params)

    key = ("nc", meta["chs_lo"], meta["chs_hi"])
    if key not in _CACHE:
        _CACHE[key] = _build(meta)
    nc = _CACHE[key]

    try:
        res = run_bass_kernel_spmd(
            nc, in_maps, core_ids=list(range(CORES)),
            trace=bool(os.environ.get("GAT_TRACE")))
    except ModuleNotFoundError:
        res = run_bass_kernel_spmd(nc, in_maps, core_ids=list(range(CORES)))
    kernel._last_result = res

    pooled = np.zeros((G_GRAPHS, G_GRAPHS), np.float64)
    for c in range(CORES):
        pooled += res.results[c]["pooled"].astype(np.float64)
    cnt = np.bincount(batch, minlength=G_GRAPHS).astype(np.float64)
    out = pooled / np.maximum(cnt, 1.0)[:, None]
    return out.astype(np.float32)


# revision 55
# speedup vs baseline: 1.0454x; 1.0091x over previous
"""GATv2 (3-layer, heads=4/4/1) full-graph kernel for 8 Trainium2 NeuronCores.

Contract: kernel(**inputs) takes the FULL unsharded inputs (as produced by
setup_inputs()) and returns the FULL [64, 64] float32 output.

v4 design (vs. v3 baseline at 1.90 ms):
- Layer-0 phase A is REPLICATED: every core computes xl0 for all 50176 nodes
  straight from the (fully available) input x and writes it to local DRAM.
  The 251 us layer-0 feature AllGather is gone entirely.
- Layers 1-2 exchange xl via CHUNKED AllGathers overlapped with compute:
  phase A(l+1) for a chunk of own tiles runs as soon as phase B(l) finishes
  those tiles, and the chunk's AllGather fires immediately, running on the
  collective cores while phase B(l) continues on later chunks.  xl_full rows
  are laid out (chunk, core, row)-major so every AllGather lands in a
  contiguous slice with IR identical on all cores.
- Gathers issue one dma_gather per (group, stream) (up to 3840 indices per
  call, 8192-descriptor SWDGE ring) to amortize the ~1 us fixed SWDGE cost.
- Pad targets (node ids >= N) get a fake self-loop so every target has a
  nonzero softmax denominator; per-tile normalization is then a single DVE
  divide (no max/reciprocal dance, no NaNs reaching the pool matmul).
- Per-edge xr is expanded on the PE from SBUF-resident xr tiles via host-built
  fp8 one-hot matrices (oh: [lane,tgt], ohT: [tgt,lane]); gathered xl rows are
  accumulated into the same PSUM via an identity matmul; leaky-relu applied
  straight from PSUM on ACT; scores via DVE mult + halving-tree; softmax
  without max-shift (scores empirically in [-8, 7]); scatter-sum + denominators
  via fp8 one-hot matmul into PSUM; global-mean-pool partials via PE, summed
  and divided on the host.
"""
import os
import numpy as np
import ml_dtypes

import concourse.bacc as bacc
import concourse.mybir as mybir
import concourse.tile as tile
from concourse._compat import get_trn_type
from concourse.bass_utils import run_bass_kernel_spmd

f16 = mybir.dt.float16
f32 = mybir.dt.float32
f8 = mybir.dt.float8e4
i16 = mybir.dt.int16
f8np = ml_dtypes.float8_e4m3

P = 128
N = 50000
E = 800000
NP_ = 50176            # padded nodes = 392 * 128
NT = NP_ // P          # 392 global tiles
CORES = 8
NTC = NT // CORES      # 49 tiles per core
NC_NODES = NTC * P     # 6272 nodes per core
HALF = NP_ // 2        # 25088 rows per shared half
G_GRAPHS = 64
NEG = 0.2
GROUP = 3              # tiles per gather/compute group
BATCH = 8              # chunks per PSUM u-batch
H_L = [4, 4, 1]
W_L = [128, 128, 64]   # xl/value width per layer
CHUNKS = [(0, 13), (13, 12), (25, 12), (37, 12)]  # (tt0, sz) ag-pipeline chunks

_CACHE = {}


def _pack_idx_image(seq):
    """int16 index sequence -> gather SBUF image [128, len/16]."""
    n = len(seq)
    assert n % 16 == 0
    img = np.asarray(seq, np.int16).reshape(n // 16, 16).T
    return np.tile(img, (8, 1))


def _chunk_groups(tt0, sz):
    out = []
    gi = tt0
    while gi < tt0 + sz:
        out.append((gi, min(GROUP, tt0 + sz - gi)))
        gi += GROUP
    return out


def _group_list():
    out = []
    for tt0, sz in CHUNKS:
        out.extend(_chunk_groups(tt0, sz))
    return out


def _tile_slot():
    """global tile t -> slot in the (chunk, core, row) xl_full layout."""
    slot = np.empty(NT, np.int64)
    for c in range(CORES):
        for tt0, sz in CHUNKS:
            for i in range(sz):
                slot[NTC * c + tt0 + i] = CORES * tt0 + c * sz + i
    return slot


def _pack_perm(h, c):
    """column permutation: packed[cw*h_n + hh] = natural[hh*c + cw]."""
    perm = np.empty(h * c, np.int64)
    for cw in range(c):
        for hh in range(h):
            perm[cw * h + hh] = hh * c + cw
    return perm


def _balance_perm(edge_index):
    """Relabel real nodes so every 128-node tile has near-equal in-degree.
    Returns perm[orig] -> new position (pads N..NP_ stay in place)."""
    import heapq
    deg = np.bincount(edge_index[1].astype(np.int64), minlength=N) + 1
    order = np.argsort(-deg, kind="stable")
    nfull = N // P                      # 390 full tiles
    caps = [P] * nfull + [N - nfull * P]  # tile 390 gets the remainder
    heap = [(0, b) for b in range(len(caps))]
    heapq.heapify(heap)
    fill = [0] * len(caps)
    perm = np.empty(N, np.int64)
    for v in order:
        while True:
            s, b = heapq.heappop(heap)
            if fill[b] < caps[b]:
                break
        perm[v] = b * P + fill[b]
        fill[b] += 1
        if fill[b] < caps[b]:
            heapq.heappush(heap, (s + int(deg[v]), b))
    return perm


def _preprocess(x, edge_index, batch, params):
    nperm = _balance_perm(edge_index)
    loops = np.arange(N, dtype=np.int64)
    pads = np.arange(N, NP_, dtype=np.int64)   # fake self-loops on pad targets
    src = np.concatenate([nperm[edge_index[0].astype(np.int64)], nperm[loops],
                          pads])
    tgt = np.concatenate([nperm[edge_index[1].astype(np.int64)], nperm[loops],
                          pads])
    order = np.argsort(tgt, kind="stable")
    srcs, tgts = src[order], tgt[order]

    slot = _tile_slot()
    src_row = slot[srcs // P] * P + srcs % P   # permuted xl_full row per edge
    islo = src_row < HALF

    bounds = np.searchsorted(tgts, np.arange(0, NP_ + 1, P))
    nlo = np.empty(NT, np.int64)
    nhi = np.empty(NT, np.int64)
    for t in range(NT):
        s, e = bounds[t], bounds[t + 1]
        nlo[t] = int(islo[s:e].sum())
        nhi[t] = (e - s) - nlo[t]
    # per-core-tile-slot chunk counts (max over cores, static across SPMD IR)
    chs_lo = tuple(
        int(max(1, -(-nlo[tt::NTC].max() // P))) for tt in range(NTC))
    chs_hi = tuple(
        int(max(1, -(-nhi[tt::NTC].max() // P))) for tt in range(NTC))

    # per-layer packed weights / attention
    wlrs, att_reps = [], []
    prev_perm = None  # input-feature permutation (packing of previous layer)
    for li, (Wl, Wr, att) in enumerate(params):
        h, c = att.shape
        hc = h * c
        Wl = np.asarray(Wl, np.float32)
        Wr = np.asarray(Wr, np.float32)
        if prev_perm is not None:
            Wl = Wl[prev_perm]
            Wr = Wr[prev_perm]
        if li < 2:
            perm = _pack_perm(h, c)
            Wl = Wl[:, perm]
            Wr = Wr[:, perm]
            att_flat = np.asarray(att, np.float32).reshape(-1)[perm]
            prev_perm = perm
        else:
            att_flat = np.asarray(att, np.float32).reshape(-1)
            prev_perm = None
        wlr = np.zeros((P, 256), np.float16)
        wlr[: Wl.shape[0], :hc] = Wl.astype(np.float16)
        wlr[: Wr.shape[0], 128 : 128 + hc] = Wr.astype(np.float16)
        wlrs.append(wlr)
        af = np.zeros(P, np.float16)
        af[:hc] = att_flat.astype(np.float16)
        att_reps.append(np.tile(af[None, :], (P, 1)))

    ident = np.eye(P).astype(f8np)

    x_pad = np.zeros((NP_, P), np.float32)
    x_pad[nperm] = np.asarray(x, np.float32)   # rows at balanced positions
    # x columns in slot-major (permuted) order, shared by all cores
    slot_tile = np.empty(NT, np.int64)
    slot_tile[slot] = np.arange(NT)
    xTp = np.empty((P, NP_), np.float16)
    for s in range(NT):
        t = slot_tile[s]
        xTp[:, s * P:(s + 1) * P] = x_pad[t * P:(t + 1) * P].astype(np.float16).T

    grp_list = _group_list()

    # graph id per NEW position (pads -> 0, masked out by valid)
    batch_perm = np.zeros(NP_, np.int64)
    batch_perm[nperm] = np.asarray(batch, np.int64)
    valid_perm = np.zeros(NP_, bool)
    valid_perm[nperm] = True

    in_maps = []
    for c in range(CORES):
        t0 = c * NTC
        base = t0 * P
        # per-(tile, stream) slot tables, padded to chs_*[tt]*128
        xlo = [np.zeros(chs_lo[tt] * P, np.int64) for tt in range(NTC)]
        xhi = [np.zeros(chs_hi[tt] * P, np.int64) for tt in range(NTC)]
        tl_lo = [np.full(chs_lo[tt] * P, -1, np.int64) for tt in range(NTC)]
        tl_hi = [np.full(chs_hi[tt] * P, -1, np.int64) for tt in range(NTC)]
        for tt in range(NTC):
            t = t0 + tt
            s, e = bounds[t], bounds[t + 1]
            sl = tgts[s:e] - t * P
            sp = src_row[s:e]
            lo_mask = islo[s:e]
            k = int(lo_mask.sum()); k2 = (e - s) - k
            xlo[tt][:k] = sp[lo_mask]
            tl_lo[tt][:k] = sl[lo_mask]
            xhi[tt][:k2] = sp[~lo_mask] - HALF
            tl_hi[tt][:k2] = sl[~lo_mask]

        # group-stream-major chunk columns
        lo_imgs, hi_imgs = [], []
        oh_cols, ohT_cols = [], []
        for gi, g in grp_list:
            lo_seq = np.concatenate(xlo[gi:gi + g])
            hi_seq = np.concatenate(xhi[gi:gi + g])
            lo_imgs.append(_pack_idx_image(lo_seq))
            hi_imgs.append(_pack_idx_image(hi_seq))
            tl_seq = np.concatenate(tl_lo[gi:gi + g] + tl_hi[gi:gi + g])
            nch2 = len(tl_seq) // P
            tl_mat = tl_seq.reshape(nch2, P)          # [chunk, lane] -> tloc
            oh = np.zeros((P, nch2, P), f8np)         # [lane, chunk, tgt]
            ohT = np.zeros((P, nch2, P), f8np)        # [tgt, chunk, lane]
            ch_i, ln_i = np.nonzero(tl_mat >= 0)
            tl_v = tl_mat[ch_i, ln_i]
            oh[ln_i, ch_i, tl_v] = 1.0
            ohT[tl_v, ch_i, ln_i] = 1.0
            oh_cols.append(oh)
            ohT_cols.append(ohT)

        # pooling one-hot [128, NTC, 64]
        pool = np.zeros((P, NTC, G_GRAPHS), np.float16)
        for tt in range(NTC):
            gn = base + tt * P + np.arange(P)
            valid = valid_perm[gn]
            pool[valid, tt, batch_perm[gn[valid]]] = 1.0

        # own-shard x columns (natural tt order) for the layer-0 xr pass
        xr0T = np.ascontiguousarray(
            x_pad[base:base + NC_NODES].astype(np.float16).T)

        in_maps.append({
            "x0T": xTp,
            "xr0T": xr0T,
            "xlidxlo": np.concatenate(lo_imgs, axis=1),
            "xlidxhi": np.concatenate(hi_imgs, axis=1),
            "oh": np.concatenate(oh_cols, axis=1),
            "ohT": np.concatenate(ohT_cols, axis=1),
            "ident": ident,
            "attr0": att_reps[0], "attr1": att_reps[1], "attr2": att_reps[2],
            "wlr0": wlrs[0], "wlr1": wlrs[1], "wlr2": wlrs[2],
            "pooloh": pool,
        })

    return dict(chs_lo=chs_lo, chs_hi=chs_hi), in_maps


def _build(meta):
    chs_lo, chs_hi = meta["chs_lo"], meta["chs_hi"]
    NIL = sum(chs_lo)   # lo chunks per core
    NIH = sum(chs_hi)
    NCH = NIL + NIH     # total chunk columns per core
    MCH = max(max(chs_lo), max(chs_hi))
    nc = bacc.Bacc(
        get_trn_type() or "TRN2",
        target_bir_lowering=False,
        debug=False,
        num_devices=CORES,
        dynamic_dma_scratch_size=32768,   # 2048-descriptor SWDGE ring
    )
    inp = {}
    for name, shape, dt in [
        ("x0T", [P, NP_], f16),
        ("xr0T", [P, NC_NODES], f16),
        ("xlidxlo", [P, NIL * 8], i16),
        ("xlidxhi", [P, NIH * 8], i16),
        ("oh", [P, NCH, P], f8),
        ("ohT", [P, NCH, P], f8),
        ("ident", [P, P], f8),
        ("attr0", [P, P], f16), ("attr1", [P, P], f16), ("attr2", [P, P], f16),
        ("wlr0", [P, 256], f16), ("wlr1", [P, 256], f16), ("wlr2", [P, 256], f16),
        ("pooloh", [P, NTC, G_GRAPHS], f16),
    ]:
        inp[name] = nc.dram_tensor(name, shape, dt, kind="ExternalInput")

    pooled = nc.dram_tensor("pooled", [G_GRAPHS, G_GRAPHS], f32,
                            kind="ExternalOutput")
    dbg = {}
    if os.environ.get("GAT_DEBUG"):
        dbg["xl0"] = nc.dram_tensor("dbg_xl0", [NP_, P], f16,
                                    kind="ExternalOutput")
        dbg["xn0"] = nc.dram_tensor("dbg_xn0", [NC_NODES, P], f16,
                                    kind="ExternalOutput")
        dbg["xl1"] = nc.dram_tensor("dbg_xl1", [NP_, P], f16,
                                    kind="ExternalOutput")
        dbg["st0"] = nc.dram_tensor("dbg_st0", [P, 64, P], f16,
                                    kind="ExternalOutput")
        dbg["L0"] = nc.dram_tensor("dbg_L0", [P, 64, P], f16,
                                   kind="ExternalOutput")
        dbg["w0"] = nc.dram_tensor("dbg_w0", [P, 64, P + 4], f16,
                                   kind="ExternalOutput")

    # xl_full[l]: per-edge gather source, rows in (chunk, core, row) slot order
    xl_full = [
        nc.dram_tensor("xl_full0", [NP_, P], f16),
        nc.dram_tensor("xl_full1", [NP_, P], f16, addr_space="Shared"),
        nc.dram_tensor("xl_full2", [NP_, P], f16),
    ]
    # layer-2 ag payload is only 64 wide; gathered rows must still be 256B,
    # so ag lands compact and a local DMA expands into xl_full2's row pitch
    xl2c = nc.dram_tensor("xl_full2c", [NP_, W_L[2]], f16, addr_space="Shared")
    w_own2 = P if os.environ.get("GAT_L2FULL") else W_L[2]
    xl_own = [
        None,
        nc.dram_tensor("xl_own1", [NC_NODES, P], f16),
        nc.dram_tensor("xl_own2", [NC_NODES, w_own2], f16),
    ]
    xn_own = nc.dram_tensor("xn_own", [NC_NODES, P], f16)

    grp_list = _group_list()
    # per-group descriptors: chunk lists per stream, cumulative offsets
    ginfo = {}
    a_lo = a_hi = a_o = 0
    for gi, g in grp_list:
        lo_list = [tt for tt in range(gi, gi + g) for _ in range(chs_lo[tt])]
        hi_list = [tt for tt in range(gi, gi + g) for _ in range(chs_hi[tt])]
        ginfo[gi] = dict(lo=lo_list, hi=hi_list, io_lo=a_lo, io_hi=a_hi,
                         oo=a_o)
        a_lo += len(lo_list) * 8
        a_hi += len(hi_list) * 8
        a_o += len(lo_list) + len(hi_list)

    SA = 8  # tiles per phase-A strip (shares the [P,8,128] psu PSUM tag)

    with tile.TileContext(nc) as tc:
        with (
            tc.tile_pool(name="const", bufs=1) as cpool,
            tc.tile_pool(name="stage", bufs=1) as spool,
            tc.tile_pool(name="strip", bufs=3) as stpool,
            tc.tile_pool(name="edge", bufs=2) as epool,
            tc.tile_pool(name="small", bufs=3) as smpool,
            tc.tile_pool(name="psU", bufs=2, space="PSUM") as psU,
            tc.tile_pool(name="psS", bufs=3, space="PSUM") as psS,
            tc.tile_pool(name="psP", bufs=1, space="PSUM") as psP,
        ):
            ident_t = cpool.tile([P, P], f8)
            nc.sync.dma_start(out=ident_t[:], in_=inp["ident"][:])
            pool_t = cpool.tile([P, NTC, G_GRAPHS], f16)
            nc.sync.dma_start(out=pool_t[:], in_=inp["pooloh"][:])
            wlr_t, att_t = [], []
            for l in range(3):
                w_t_ = cpool.tile([P, 256], f16, tag=f"wlr{l}")
                nc.sync.dma_start(out=w_t_[:], in_=inp[f"wlr{l}"][:])
                wlr_t.append(w_t_)
                a_t_ = cpool.tile([P, P], f16, tag=f"att{l}")
                nc.sync.dma_start(out=a_t_[:], in_=inp[f"attr{l}"][:])
                att_t.append(a_t_)

            pool_psum = psP.tile([G_GRAPHS, G_GRAPHS], f32, space="PSUM")

            # persistent per-layer state
            xr_sb = spool.tile([P, NTC, P], f16, tag="xr_sb")
            stg_xl = spool.tile([P, NTC, P], f16, tag="stg_xl")
            stg_xn = spool.tile([P, NTC, P], f16, tag="stg_xn")

            ncopy = [0]

            def psum_copy(dst, src):
                # alternate PSUM->SBUF copies between ACT and DVE
                if ncopy[0] % 2 == 0:
                    nc.scalar.copy(out=dst, in_=src)
                else:
                    nc.vector.tensor_copy(out=dst, in_=src)
                ncopy[0] += 1

            # ---- replicated phase A, layer 0: xl0 for ALL slots ----
            for s0 in range(0, NT, SA):
                w_ = min(SA, NT - s0)
                xs_t = stpool.tile([P, SA * P], f16, tag="xstrip")
                nc.sync.dma_start(out=xs_t[:, :w_ * P],
                                  in_=inp["x0T"][:, s0 * P:(s0 + w_) * P])
                ps = psU.tile([P, SA, P], f32, space="PSUM", tag="psu")
                for j in range(w_):
                    nc.tensor.matmul(
                        out=ps[:, j, :], lhsT=xs_t[:, j * P:(j + 1) * P],
                        rhs=wlr_t[0][:, :P], start=True, stop=True)
                stg = stpool.tile([P, SA, P], f16, tag="a0stg")
                psum_copy(stg[:, :w_, :], ps[:, :w_, :])
                nc.sync.dma_start(
                    out=xl_full[0][s0 * P:(s0 + w_) * P, :].rearrange(
                        "(t p) f -> p t f", p=P),
                    in_=stg[:, :w_, :])
            # layer-0 xr for own tiles
            for s0 in range(0, NTC, SA):
                w_ = min(SA, NTC - s0)
                xs_t = stpool.tile([P, SA * P], f16, tag="xstrip")
                nc.sync.dma_start(out=xs_t[:, :w_ * P],
                                  in_=inp["xr0T"][:, s0 * P:(s0 + w_) * P])
                ps = psU.tile([P, SA, P], f32, space="PSUM", tag="psu")
                for j in range(w_):
                    nc.tensor.matmul(
                        out=ps[:, j, :], lhsT=xs_t[:, j * P:(j + 1) * P],
                        rhs=wlr_t[0][:, 128:256], start=True, stop=True)
                psum_copy(xr_sb[:, s0:s0 + w_, :], ps[:, :w_, :])

            # ---- layers ----
            for l in range(3):
                Hh = H_L[l]
                W = W_L[l]
                CW = W // Hh
                att_l = att_t[l]

                for tt0, sz in CHUNKS:
                    for gi, g in _chunk_groups(tt0, sz):
                        info = ginfo[gi]
                        lo_list, hi_list = info["lo"], info["hi"]
                        nch_lo, nch_hi = len(lo_list), len(hi_list)
                        nch2 = nch_lo + nch_hi
                        col0 = info["oo"]

                        ilo = smpool.tile([P, GROUP * MCH * 8], i16, tag="ilo")
                        nc.sync.dma_start(
                            out=ilo[:, :nch_lo * 8],
                            in_=inp["xlidxlo"][
                                :, info["io_lo"]:info["io_lo"] + nch_lo * 8])
                        ihi = smpool.tile([P, GROUP * MCH * 8], i16, tag="ihi")
                        nc.sync.dma_start(
                            out=ihi[:, :nch_hi * 8],
                            in_=inp["xlidxhi"][
                                :, info["io_hi"]:info["io_hi"] + nch_hi * 8])
                        oh_t = epool.tile([P, 2 * GROUP * MCH, P], f8,
                                          tag="oh")
                        nc.sync.dma_start(
                            out=oh_t[:, :nch2, :],
                            in_=inp["oh"][:, col0:col0 + nch2, :])
                        ohT_t = epool.tile([P, 2 * GROUP * MCH, P], f8,
                                           tag="ohT")
                        for o0 in range(0, nch2, BATCH):
                            on = min(BATCH, nch2 - o0)
                            nc.sync.dma_start(
                                out=ohT_t[:, o0:o0 + on, :],
                                in_=inp["ohT"][:, col0 + o0:col0 + o0 + on, :])

                        # gather calls (<= GMAX idxs each) per stream;
                        # GMAX must stay <= half the SWDGE ring (2048 descs)
                        GMAX = 1024
                        def gathers(st, in_ap, idx_t, nch_s):
                            k = 0
                            while k < nch_s * P:
                                n = min(GMAX, nch_s * P - k)
                                nc.gpsimd.dma_gather(
                                    out_ap=st[:, k // P:(k + n) // P, :],
                                    in_ap=in_ap,
                                    idxs_ap=idx_t[:, k // 16:(k + n) // 16],
                                    num_idxs=n, num_idxs_reg=n, elem_size=P)
                                k += n
                        st_lo = epool.tile([P, GROUP * MCH, P], f16,
                                           tag="xlglo")
                        gathers(st_lo, xl_full[l][0:HALF, :], ilo, nch_lo)
                        st_hi = epool.tile([P, GROUP * MCH, P], f16,
                                           tag="xlghi")
                        gathers(st_hi, xl_full[l][HALF:NP_, :], ihi, nch_hi)

                        # u = xr[tloc] + xl_src  (PSUM), leaky-relu -> L
                        L_t = epool.tile([P, 2 * GROUP * MCH, P], f16, tag="L")
                        for c0, xt, clist in ((0, st_lo, lo_list),
                                              (nch_lo, st_hi, hi_list)):
                            for b0 in range(0, len(clist), BATCH):
                                nb = min(BATCH, len(clist) - b0)
                                psu = psU.tile([P, BATCH, P], f32,
                                               space="PSUM", tag="psu")
                                for k in range(nb):
                                    cc = b0 + k
                                    nc.tensor.matmul(
                                        out=psu[:, k, :W],
                                        lhsT=ohT_t[:, c0 + cc, :],
                                        rhs=xr_sb[:, clist[cc], :W],
                                        start=True, stop=False)
                                    nc.tensor.matmul(
                                        out=psu[:, k, :W], lhsT=ident_t[:],
                                        rhs=xt[:, cc, :W],
                                        start=False, stop=True)
                                nc.scalar.activation(
                                    out=L_t[:, c0 + b0:c0 + b0 + nb, :W],
                                    in_=psu[:, :nb, :W],
                                    func=mybir.ActivationFunctionType.Prelu,
                                    alpha=NEG)

                        # scores: L *= att ; tree-reduce over cw
                        nc.vector.tensor_tensor(
                            out=L_t[:, :nch2, :W], in0=L_t[:, :nch2, :W],
                            in1=att_l[:, :W].unsqueeze(1).broadcast_to(
                                [P, nch2, W]),
                            op=mybir.AluOpType.mult)
                        w_t = epool.tile([P, 2 * GROUP * MCH, P + 4], f16,
                                         tag="w")
                        Lv = L_t[:, :nch2, :W].rearrange(
                            "p c (w h) -> p c w h", h=Hh)
                        # tree halves in place into L_t's low columns (L is
                        # dead after the att-mult; keeps w_t alias-free)
                        tv = L_t[:, :nch2, :W // 2].rearrange(
                            "p c (w h) -> p c w h", h=Hh)
                        half = CW // 2
                        nc.vector.tensor_tensor(
                            out=tv[:, :, :half, :], in0=Lv[:, :, :half, :],
                            in1=Lv[:, :, half:, :], op=mybir.AluOpType.add)
                        while half > 1:
                            q = half // 2
                            nc.vector.tensor_tensor(
                                out=tv[:, :, :q, :], in0=tv[:, :, :q, :],
                                in1=tv[:, :, q:half, :], op=mybir.AluOpType.add)
                            half = q
                        # w values and alpha
                        nc.scalar.activation(
                            out=w_t[:, :nch2, W:W + Hh], in_=tv[:, :nch2, 0, :],
                            func=mybir.ActivationFunctionType.Exp)
                        a_b = w_t[:, :nch2, W:W + Hh].unsqueeze(2).broadcast_to(
                            [P, nch2, CW, Hh])
                        for c0, xt, clist in ((0, st_lo, lo_list),
                                              (nch_lo, st_hi, hi_list)):
                            ns = len(clist)
                            nc.vector.tensor_tensor(
                                out=w_t[:, c0:c0 + ns, :W].rearrange(
                                    "p c (w h) -> p c w h", h=Hh),
                                in0=xt[:, :ns, :W].rearrange(
                                    "p c (w h) -> p c w h", h=Hh),
                                in1=a_b[:, c0:c0 + ns],
                                op=mybir.AluOpType.mult)

                        if dbg and l == 0 and gi == 0:
                            nc.sync.dma_start(out=dbg["st0"][:, :nch_lo, :],
                                              in_=st_lo[:, :nch_lo, :])
                            nc.sync.dma_start(out=dbg["L0"][:, :nch2, :],
                                              in_=L_t[:, :nch2, :])
                            nc.sync.dma_start(out=dbg["w0"][:, :nch2, :],
                                              in_=w_t[:, :nch2, :])

                        # scatter per tile
                        for tt_ in range(g):
                            t = gi + tt_
                            cids = ([c0 for c0, tt in enumerate(lo_list)
                                     if tt == t]
                                    + [nch_lo + c0
                                       for c0, tt in enumerate(hi_list)
                                       if tt == t])
                            ps = psS.tile([P, P + 4], f32, space="PSUM",
                                          tag="pss")
                            for cix, cid in enumerate(cids):
                                nc.tensor.matmul(
                                    out=ps[:, :W + Hh],
                                    lhsT=oh_t[:, cid, :],
                                    rhs=w_t[:, cid, :W + Hh],
                                    start=(cix == 0),
                                    stop=(cix == len(cids) - 1))
                            rec = smpool.tile([P, 4], f32, tag="rec")
                            nc.vector.reciprocal(out=rec[:, :Hh],
                                                 in_=ps[:, W:W + Hh])
                            t1 = smpool.tile([P, P], f16, tag="t1")
                            nc.vector.tensor_tensor(
                                out=t1[:, :W].rearrange(
                                    "p (w h) -> p w h", h=Hh),
                                in0=ps[:, :W].rearrange(
                                    "p (w h) -> p w h", h=Hh),
                                in1=rec[:, :Hh].unsqueeze(1).broadcast_to(
                                    [P, CW, Hh]),
                                op=mybir.AluOpType.mult)
                            if l < 2:
                                nc.scalar.activation(
                                    out=stg_xn[:, t, :], in_=t1[:],
                                    func=mybir.ActivationFunctionType.Prelu,
                                    alpha=NEG)
                            else:
                                xnm = smpool.tile([P, G_GRAPHS], f16,
                                                  tag="xnm2")
                                nc.scalar.activation(
                                    out=xnm[:], in_=t1[:, :G_GRAPHS],
                                    func=mybir.ActivationFunctionType.Prelu,
                                    alpha=NEG)
                                nc.tensor.matmul(
                                    out=pool_psum[:],
                                    lhsT=pool_t[:, t, :], rhs=xnm[:],
                                    start=(t == 0), stop=(t == NTC - 1))

                    # ---- interleaved phase A(l+1) + chunk AllGather ----
                    if l < 2:
                        Wn = W_L[l + 1]
                        nc.sync.dma_start(
                            out=xn_own[tt0 * P:(tt0 + sz) * P, :].rearrange(
                                "(t p) f -> p t f", p=P),
                            in_=stg_xn[:, tt0:tt0 + sz, :])
                        for j0 in range(tt0, tt0 + sz, 2):
                            w_ = min(2, tt0 + sz - j0)
                            xs_t = stpool.tile([P, 2 * P], f16, tag="xstrip2")
                            nc.sync.dma_start_transpose(
                                out=xs_t[:, :w_ * P],
                                in_=xn_own[j0 * P:(j0 + w_) * P, :])
                            ps = psU.tile([P, SA, P], f32, space="PSUM",
                                          tag="psu")
                            for j in range(w_):
                                nc.tensor.matmul(
                                    out=ps[:, j, :Wn],
                                    lhsT=xs_t[:, j * P:(j + 1) * P],
                                    rhs=wlr_t[l + 1][:, :Wn],
                                    start=True, stop=True)
                                nc.tensor.matmul(
                                    out=ps[:, 4 + j, :Wn],
                                    lhsT=xs_t[:, j * P:(j + 1) * P],
                                    rhs=wlr_t[l + 1][:, 128:128 + Wn],
                                    start=True, stop=True)
                            psum_copy(stg_xl[:, j0:j0 + w_, :Wn],
                                      ps[:, :w_, :Wn])
                            psum_copy(xr_sb[:, j0:j0 + w_, :Wn],
                                      ps[:, 4:4 + w_, :Wn])
                        Ws = xl_own[l + 1].shape[1]
                        nc.sync.dma_start(
                            out=xl_own[l + 1][tt0 * P:(tt0 + sz) * P, :]
                            .rearrange("(t p) f -> p t f", p=P),
                            in_=stg_xl[:, tt0:tt0 + sz, :Ws])
                        s0 = CORES * tt0 * P
                        s1 = CORES * (tt0 + sz) * P
                        if Wn == P or os.environ.get("GAT_L2FULL"):
                            nc.gpsimd.collective_compute(
                                "AllGather", mybir.AluOpType.bypass,
                                replica_groups=[list(range(CORES))],
                                ins=[xl_own[l + 1][tt0 * P:(tt0 + sz) * P, :]],
                                outs=[xl_full[l + 1][s0:s1, :]])
                        else:
                            nc.gpsimd.collective_compute(
                                "AllGather", mybir.AluOpType.bypass,
                                replica_groups=[list(range(CORES))],
                                ins=[xl_own[l + 1][tt0 * P:(tt0 + sz) * P, :]],
                                outs=[xl2c[s0:s1, :]])
                            nc.sync.dma_start(
                                out=xl_full[l + 1][s0:s1, :Wn],
                                in_=xl2c[s0:s1, :])

                if dbg and l == 0:
                    nc.sync.dma_start(out=dbg["xl0"][:], in_=xl_full[0][:])
                    nc.sync.dma_start(out=dbg["xn0"][:], in_=xn_own[:])
                if dbg and l == 1:
                    nc.sync.dma_start(out=dbg["xl1"][:], in_=xl_full[1][:])

            pool_sb = smpool.tile([G_GRAPHS, G_GRAPHS], f32, tag="poolsb")
            nc.vector.tensor_copy(out=pool_sb[:], in_=pool_psum[:])
            nc.sync.dma_start(out=pooled[:], in_=pool_sb[:])

    nc.finalize()
    return nc


def kernel(**inputs):
    x = np.asarray(inputs["x"])
    edge_index = np.asarray(inputs["edge_index"])
    batch = np.asarray(inputs["batch"])
    params = []
    for l in range(3):
        params.append((np.asarray(inputs[f"Wl{l}"]),
                       np.asarray(inputs[f"Wr{l}"]),
                       np.asarray(inputs[f"att{l}"])))
        b = np.asarray(inputs[f"b{l}"])
        assert np.all(b == 0), "nonzero bias not supported"

    meta, in_maps = _preprocess(x, edge_index, batch, params)

    key = ("nc", meta["chs_lo"], meta["chs_hi"])
    if key not in _CACHE:
        _CACHE[key] = _build(meta)
    nc = _CACHE[key]

    try:
        res = run_bass_kernel_spmd(
            nc, in_maps, core_ids=list(range(CORES)),
            trace=bool(os.environ.get("GAT_TRACE")))
    except ModuleNotFoundError:
        res = run_bass_kernel_spmd(nc, in_maps, core_ids=list(range(CORES)))
    kernel._last_result = res

    pooled = np.zeros((G_GRAPHS, G_GRAPHS), np.float64)
    for c in range(CORES):
        pooled += res.results[c]["pooled"].astype(np.float64)
    cnt = np.bincount(batch, minlength=G_GRAPHS).astype(np.float64)
    out = pooled / np.maximum(cnt, 1.0)[:, None]
    return out.astype(np.float32)


# revision 63
# speedup vs baseline: 1.0646x; 1.0184x over previous
"""GATv2 (3-layer, heads=4/4/1) full-graph kernel for 8 Trainium2 NeuronCores.

Contract: kernel(**inputs) takes the FULL unsharded inputs (as produced by
setup_inputs()) and returns the FULL [64, 64] float32 output.

v4 design (vs. v3 baseline at 1.90 ms):
- Layer-0 phase A is REPLICATED: every core computes xl0 for all 50176 nodes
  straight from the (fully available) input x and writes it to local DRAM.
  The 251 us layer-0 feature AllGather is gone entirely.
- Layers 1-2 exchange xl via CHUNKED AllGathers overlapped with compute:
  phase A(l+1) for a chunk of own tiles runs as soon as phase B(l) finishes
  those tiles, and the chunk's AllGather fires immediately, running on the
  collective cores while phase B(l) continues on later chunks.  xl_full rows
  are laid out (chunk, core, row)-major so every AllGather lands in a
  contiguous slice with IR identical on all cores.
- Per-edge xl gathers run as 1024-index dma_gather calls (half the
  2048-descriptor SWDGE ring, so two stay in flight); nodes are relabeled by
  a degree-balancing permutation and chunk counts are sized per tile slot
  (max over cores) instead of one global worst case.
- Pad targets (node ids >= N) get a fake self-loop so every target has a
  nonzero softmax denominator; per-tile normalization is then a single DVE
  divide (no max/reciprocal dance, no NaNs reaching the pool matmul).
- Per-edge xr is expanded on the PE from SBUF-resident xr tiles via host-built
  fp8 one-hot matrices (oh: [lane,tgt], ohT: [tgt,lane]); gathered xl rows are
  accumulated into the same PSUM via an identity matmul; leaky-relu applied
  straight from PSUM on ACT; scores via DVE mult + halving-tree; softmax
  without max-shift (scores empirically in [-8, 7]); scatter-sum + denominators
  via fp8 one-hot matmul into PSUM; global-mean-pool partials via PE, summed
  and divided on the host.
"""
import os
import numpy as np
import ml_dtypes

import concourse.bacc as bacc
import concourse.mybir as mybir
import concourse.tile as tile
from concourse._compat import get_trn_type
from concourse.bass_utils import run_bass_kernel_spmd

f16 = mybir.dt.float16
f32 = mybir.dt.float32
f8 = mybir.dt.float8e4
i16 = mybir.dt.int16
f8np = ml_dtypes.float8_e4m3

P = 128
N = 50000
E = 800000
NP_ = 50176            # padded nodes = 392 * 128
NT = NP_ // P          # 392 global tiles
CORES = 8
NTC = NT // CORES      # 49 tiles per core
NC_NODES = NTC * P     # 6272 nodes per core
HALF = NP_ // 2        # 25088 rows per shared half
G_GRAPHS = 64
NEG = 0.2
GROUP = 2              # tiles per gather/compute group
BATCH = 8              # chunks per PSUM u-batch
H_L = [4, 4, 1]
W_L = [128, 128, 64]   # xl/value width per layer
CHUNKS = [(0, 13), (13, 12), (25, 12), (37, 12)]  # (tt0, sz) ag-pipeline chunks

_CACHE = {}


def _pack_idx_image(seq):
    """int16 index sequence -> gather SBUF image [128, len/16]."""
    n = len(seq)
    assert n % 16 == 0
    img = np.asarray(seq, np.int16).reshape(n // 16, 16).T
    return np.tile(img, (8, 1))


def _chunk_groups(tt0, sz):
    out = []
    gi = tt0
    while gi < tt0 + sz:
        out.append((gi, min(GROUP, tt0 + sz - gi)))
        gi += GROUP
    return out


def _group_list():
    out = []
    for tt0, sz in CHUNKS:
        out.extend(_chunk_groups(tt0, sz))
    return out


def _tile_slot():
    """global tile t -> slot in the (chunk, core, row) xl_full layout."""
    slot = np.empty(NT, np.int64)
    for c in range(CORES):
        for tt0, sz in CHUNKS:
            for i in range(sz):
                slot[NTC * c + tt0 + i] = CORES * tt0 + c * sz + i
    return slot


def _pack_perm(h, c):
    """column permutation: packed[cw*h_n + hh] = natural[hh*c + cw]."""
    perm = np.empty(h * c, np.int64)
    for cw in range(c):
        for hh in range(h):
            perm[cw * h + hh] = hh * c + cw
    return perm


def _balance_perm(edge_index):
    """Relabel real nodes so every 128-node tile has near-equal in-degree.
    Returns perm[orig] -> new position (pads N..NP_ stay in place)."""
    import heapq
    deg = np.bincount(edge_index[1].astype(np.int64), minlength=N) + 1
    order = np.argsort(-deg, kind="stable")
    nfull = N // P                      # 390 full tiles
    caps = [P] * nfull + [N - nfull * P]  # tile 390 gets the remainder
    heap = [(0, b) for b in range(len(caps))]
    heapq.heapify(heap)
    fill = [0] * len(caps)
    perm = np.empty(N, np.int64)
    for v in order:
        while True:
            s, b = heapq.heappop(heap)
            if fill[b] < caps[b]:
                break
        perm[v] = b * P + fill[b]
        fill[b] += 1
        if fill[b] < caps[b]:
            heapq.heappush(heap, (s + int(deg[v]), b))
    return perm


def _preprocess(x, edge_index, batch, params):
    nperm = _balance_perm(edge_index)
    loops = np.arange(N, dtype=np.int64)
    pads = np.arange(N, NP_, dtype=np.int64)   # fake self-loops on pad targets
    src = np.concatenate([nperm[edge_index[0].astype(np.int64)], nperm[loops],
                          pads])
    tgt = np.concatenate([nperm[edge_index[1].astype(np.int64)], nperm[loops],
                          pads])
    order = np.argsort(tgt, kind="stable")
    srcs, tgts = src[order], tgt[order]

    slot = _tile_slot()
    src_row = slot[srcs // P] * P + srcs % P   # permuted xl_full row per edge
    islo = src_row < HALF

    bounds = np.searchsorted(tgts, np.arange(0, NP_ + 1, P))
    nlo = np.empty(NT, np.int64)
    nhi = np.empty(NT, np.int64)
    for t in range(NT):
        s, e = bounds[t], bounds[t + 1]
        nlo[t] = int(islo[s:e].sum())
        nhi[t] = (e - s) - nlo[t]
    # per-core-tile-slot chunk counts (max over cores, static across SPMD IR)
    chs_lo = tuple(
        int(max(1, -(-nlo[tt::NTC].max() // P))) for tt in range(NTC))
    chs_hi = tuple(
        int(max(1, -(-nhi[tt::NTC].max() // P))) for tt in range(NTC))

    # per-layer packed weights / attention
    wlrs, att_reps = [], []
    prev_perm = None  # input-feature permutation (packing of previous layer)
    for li, (Wl, Wr, att) in enumerate(params):
        h, c = att.shape
        hc = h * c
        Wl = np.asarray(Wl, np.float32)
        Wr = np.asarray(Wr, np.float32)
        if prev_perm is not None:
            Wl = Wl[prev_perm]
            Wr = Wr[prev_perm]
        if li < 2:
            perm = _pack_perm(h, c)
            Wl = Wl[:, perm]
            Wr = Wr[:, perm]
            att_flat = np.asarray(att, np.float32).reshape(-1)[perm]
            prev_perm = perm
        else:
            att_flat = np.asarray(att, np.float32).reshape(-1)
            prev_perm = None
        wlr = np.zeros((P, 256), np.float16)
        wlr[: Wl.shape[0], :hc] = Wl.astype(np.float16)
        wlr[: Wr.shape[0], 128 : 128 + hc] = Wr.astype(np.float16)
        wlrs.append(wlr)
        af = np.zeros(P, np.float16)
        af[:hc] = att_flat.astype(np.float16)
        att_reps.append(np.tile(af[None, :], (P, 1)))

    ident = np.eye(P).astype(f8np)

    x_pad = np.zeros((NP_, P), np.float32)
    x_pad[nperm] = np.asarray(x, np.float32)   # rows at balanced positions
    # x columns in slot-major (permuted) order, shared by all cores
    slot_tile = np.empty(NT, np.int64)
    slot_tile[slot] = np.arange(NT)
    xTp = np.empty((P, NP_), np.float16)
    for s in range(NT):
        t = slot_tile[s]
        xTp[:, s * P:(s + 1) * P] = x_pad[t * P:(t + 1) * P].astype(np.float16).T

    grp_list = _group_list()

    # graph id per NEW position (pads -> 0, masked out by valid)
    batch_perm = np.zeros(NP_, np.int64)
    batch_perm[nperm] = np.asarray(batch, np.int64)
    valid_perm = np.zeros(NP_, bool)
    valid_perm[nperm] = True

    in_maps = []
    for c in range(CORES):
        t0 = c * NTC
        base = t0 * P
        # per-(tile, stream) slot tables, padded to chs_*[tt]*128
        xlo = [np.zeros(chs_lo[tt] * P, np.int64) for tt in range(NTC)]
        xhi = [np.zeros(chs_hi[tt] * P, np.int64) for tt in range(NTC)]
        tl_lo = [np.full(chs_lo[tt] * P, -1, np.int64) for tt in range(NTC)]
        tl_hi = [np.full(chs_hi[tt] * P, -1, np.int64) for tt in range(NTC)]
        for tt in range(NTC):
            t = t0 + tt
            s, e = bounds[t], bounds[t + 1]
            sl = tgts[s:e] - t * P
            sp = src_row[s:e]
            lo_mask = islo[s:e]
            k = int(lo_mask.sum()); k2 = (e - s) - k
            xlo[tt][:k] = sp[lo_mask]
            tl_lo[tt][:k] = sl[lo_mask]
            xhi[tt][:k2] = sp[~lo_mask] - HALF
            tl_hi[tt][:k2] = sl[~lo_mask]

        # group-stream-major chunk columns
        lo_imgs, hi_imgs = [], []
        oh_cols, ohT_cols = [], []
        for gi, g in grp_list:
            lo_seq = np.concatenate(xlo[gi:gi + g])
            hi_seq = np.concatenate(xhi[gi:gi + g])
            lo_imgs.append(_pack_idx_image(lo_seq))
            hi_imgs.append(_pack_idx_image(hi_seq))
            tl_seq = np.concatenate(tl_lo[gi:gi + g] + tl_hi[gi:gi + g])
            nch2 = len(tl_seq) // P
            tl_mat = tl_seq.reshape(nch2, P)          # [chunk, lane] -> tloc
            oh = np.zeros((P, nch2, P), f8np)         # [lane, chunk, tgt]
            ohT = np.zeros((P, nch2, P), f8np)        # [tgt, chunk, lane]
            ch_i, ln_i = np.nonzero(tl_mat >= 0)
            tl_v = tl_mat[ch_i, ln_i]
            oh[ln_i, ch_i, tl_v] = 1.0
            ohT[tl_v, ch_i, ln_i] = 1.0
            oh_cols.append(oh)
            ohT_cols.append(ohT)

        # pooling one-hot [128, NTC, 64]
        pool = np.zeros((P, NTC, G_GRAPHS), np.float16)
        for tt in range(NTC):
            gn = base + tt * P + np.arange(P)
            valid = valid_perm[gn]
            pool[valid, tt, batch_perm[gn[valid]]] = 1.0

        # own-shard x columns (natural tt order) for the layer-0 xr pass
        xr0T = np.ascontiguousarray(
            x_pad[base:base + NC_NODES].astype(np.float16).T)

        in_maps.append({
            "x0T": xTp,
            "xr0T": xr0T,
            "xlidxlo": np.concatenate(lo_imgs, axis=1),
            "xlidxhi": np.concatenate(hi_imgs, axis=1),
            "oh": np.concatenate(oh_cols, axis=1),
            "ohT": np.concatenate(ohT_cols, axis=1),
            "ident": ident,
            "attr0": att_reps[0], "attr1": att_reps[1], "attr2": att_reps[2],
            "wlr0": wlrs[0], "wlr1": wlrs[1], "wlr2": wlrs[2],
            "pooloh": pool,
        })

    return dict(chs_lo=chs_lo, chs_hi=chs_hi), in_maps


def _build(meta):
    chs_lo, chs_hi = meta["chs_lo"], meta["chs_hi"]
    NIL = sum(chs_lo)   # lo chunks per core
    NIH = sum(chs_hi)
    NCH = NIL + NIH     # total chunk columns per core
    MCH = max(max(chs_lo), max(chs_hi))
    nc = bacc.Bacc(
        get_trn_type() or "TRN2",
        target_bir_lowering=False,
        debug=False,
        num_devices=CORES,
        dynamic_dma_scratch_size=32768,   # 2048-descriptor SWDGE ring
    )
    inp = {}
    for name, shape, dt in [
        ("x0T", [P, NP_], f16),
        ("xr0T", [P, NC_NODES], f16),
        ("xlidxlo", [P, NIL * 8], i16),
        ("xlidxhi", [P, NIH * 8], i16),
        ("oh", [P, NCH, P], f8),
        ("ohT", [P, NCH, P], f8),
        ("ident", [P, P], f8),
        ("attr0", [P, P], f16), ("attr1", [P, P], f16), ("attr2", [P, P], f16),
        ("wlr0", [P, 256], f16), ("wlr1", [P, 256], f16), ("wlr2", [P, 256], f16),
        ("pooloh", [P, NTC, G_GRAPHS], f16),
    ]:
        inp[name] = nc.dram_tensor(name, shape, dt, kind="ExternalInput")

    pooled = nc.dram_tensor("pooled", [G_GRAPHS, G_GRAPHS], f32,
                            kind="ExternalOutput")
    dbg = {}
    if os.environ.get("GAT_DEBUG"):
        dbg["xl0"] = nc.dram_tensor("dbg_xl0", [NP_, P], f16,
                                    kind="ExternalOutput")
        dbg["xn0"] = nc.dram_tensor("dbg_xn0", [NC_NODES, P], f16,
                                    kind="ExternalOutput")
        dbg["xl1"] = nc.dram_tensor("dbg_xl1", [NP_, P], f16,
                                    kind="ExternalOutput")
        dbg["st0"] = nc.dram_tensor("dbg_st0", [P, 64, P], f16,
                                    kind="ExternalOutput")
        dbg["L0"] = nc.dram_tensor("dbg_L0", [P, 64, P], f16,
                                   kind="ExternalOutput")
        dbg["w0"] = nc.dram_tensor("dbg_w0", [P, 64, P + 4], f16,
                                   kind="ExternalOutput")

    # xl_full[l]: per-edge gather source, rows in (chunk, core, row) slot order
    # layer 0 is split in halves so each gather stream only waits for its
    # half of the replicated phase-A sweep
    xl0lo = nc.dram_tensor("xl_full0lo", [HALF, P], f16)
    xl0hi = nc.dram_tensor("xl_full0hi", [NP_ - HALF, P], f16)
    xl_full = [
        None,
        nc.dram_tensor("xl_full1", [NP_, P], f16, addr_space="Shared"),
        nc.dram_tensor("xl_full2", [NP_, P], f16),
    ]
    # layer-2 ag payload is only 64 wide; gathered rows must still be 256B,
    # so ag lands compact and a local DMA expands into xl_full2's row pitch
    xl2c = nc.dram_tensor("xl_full2c", [NP_, W_L[2]], f16, addr_space="Shared")
    w_own2 = P if os.environ.get("GAT_L2FULL") else W_L[2]
    xl_own = [
        None,
        nc.dram_tensor("xl_own1", [NC_NODES, P], f16),
        nc.dram_tensor("xl_own2", [NC_NODES, w_own2], f16),
    ]
    xn_own = nc.dram_tensor("xn_own", [NC_NODES, P], f16)

    grp_list = _group_list()
    # per-group descriptors: chunk lists per stream, cumulative offsets
    ginfo = {}
    a_lo = a_hi = a_o = 0
    for gi, g in grp_list:
        lo_list = [tt for tt in range(gi, gi + g) for _ in range(chs_lo[tt])]
        hi_list = [tt for tt in range(gi, gi + g) for _ in range(chs_hi[tt])]
        ginfo[gi] = dict(lo=lo_list, hi=hi_list, io_lo=a_lo, io_hi=a_hi,
                         oo=a_o)
        a_lo += len(lo_list) * 8
        a_hi += len(hi_list) * 8
        a_o += len(lo_list) + len(hi_list)

    SA = 8  # tiles per phase-A strip (shares the [P,8,128] psu PSUM tag)

    with tile.TileContext(nc) as tc:
        with (
            tc.tile_pool(name="const", bufs=1) as cpool,
            tc.tile_pool(name="stage", bufs=1) as spool,
            tc.tile_pool(name="strip", bufs=3) as stpool,
            tc.tile_pool(name="edge", bufs=3) as epool,
            tc.tile_pool(name="small", bufs=3) as smpool,
            tc.tile_pool(name="psU", bufs=2, space="PSUM") as psU,
            tc.tile_pool(name="psS", bufs=3, space="PSUM") as psS,
            tc.tile_pool(name="psP", bufs=1, space="PSUM") as psP,
        ):
            ident_t = cpool.tile([P, P], f8)
            nc.sync.dma_start(out=ident_t[:], in_=inp["ident"][:])
            pool_t = cpool.tile([P, NTC, G_GRAPHS], f16)
            nc.sync.dma_start(out=pool_t[:], in_=inp["pooloh"][:])
            wlr_t, att_t = [], []
            for l in range(3):
                w_t_ = cpool.tile([P, 256], f16, tag=f"wlr{l}")
                nc.sync.dma_start(out=w_t_[:], in_=inp[f"wlr{l}"][:])
                wlr_t.append(w_t_)
                a_t_ = cpool.tile([P, P], f16, tag=f"att{l}")
                nc.sync.dma_start(out=a_t_[:], in_=inp[f"attr{l}"][:])
                att_t.append(a_t_)

            pool_psum = psP.tile([G_GRAPHS, G_GRAPHS], f32, space="PSUM")

            # persistent per-layer state
            xr_sb = spool.tile([P, NTC, P], f16, tag="xr_sb")
            stg_xl = spool.tile([P, NTC, P], f16, tag="stg_xl")
            stg_xn = spool.tile([P, NTC, P], f16, tag="stg_xn")

            ncopy = [0]

            def psum_copy(dst, src):
                # alternate PSUM->SBUF copies between ACT and DVE
                if ncopy[0] % 2 == 0:
                    nc.scalar.copy(out=dst, in_=src)
                else:
                    nc.vector.tensor_copy(out=dst, in_=src)
                ncopy[0] += 1

            # ---- replicated phase A, layer 0: xl0 for ALL slots ----
            for s0 in range(0, NT, SA):
                w_ = min(SA, NT - s0)
                xs_t = stpool.tile([P, SA * P], f16, tag="xstrip")
                nc.sync.dma_start(out=xs_t[:, :w_ * P],
                                  in_=inp["x0T"][:, s0 * P:(s0 + w_) * P])
                ps = psU.tile([P, SA, P], f32, space="PSUM", tag="psu")
                for j in range(w_):
                    nc.tensor.matmul(
                        out=ps[:, j, :], lhsT=xs_t[:, j * P:(j + 1) * P],
                        rhs=wlr_t[0][:, :P], start=True, stop=True)
                stg = stpool.tile([P, SA, P], f16, tag="a0stg")
                psum_copy(stg[:, :w_, :], ps[:, :w_, :])
                r0, r1 = s0 * P, (s0 + w_) * P
                HS = HALF // P
                if s0 + w_ <= HS:
                    nc.sync.dma_start(
                        out=xl0lo[r0:r1, :].rearrange("(t p) f -> p t f", p=P),
                        in_=stg[:, :w_, :])
                elif s0 >= HS:
                    nc.sync.dma_start(
                        out=xl0hi[r0 - HALF:r1 - HALF, :].rearrange(
                            "(t p) f -> p t f", p=P),
                        in_=stg[:, :w_, :])
                else:
                    k = HS - s0
                    nc.sync.dma_start(
                        out=xl0lo[r0:HALF, :].rearrange("(t p) f -> p t f", p=P),
                        in_=stg[:, :k, :])
                    nc.sync.dma_start(
                        out=xl0hi[0:r1 - HALF, :].rearrange(
                            "(t p) f -> p t f", p=P),
                        in_=stg[:, k:w_, :])
            # layer-0 xr for own tiles
            for s0 in range(0, NTC, SA):
                w_ = min(SA, NTC - s0)
                xs_t = stpool.tile([P, SA * P], f16, tag="xstrip")
                nc.sync.dma_start(out=xs_t[:, :w_ * P],
                                  in_=inp["xr0T"][:, s0 * P:(s0 + w_) * P])
                ps = psU.tile([P, SA, P], f32, space="PSUM", tag="psu")
                for j in range(w_):
                    nc.tensor.matmul(
                        out=ps[:, j, :], lhsT=xs_t[:, j * P:(j + 1) * P],
                        rhs=wlr_t[0][:, 128:256], start=True, stop=True)
                psum_copy(xr_sb[:, s0:s0 + w_, :], ps[:, :w_, :])

            # ---- layers ----
            for l in range(3):
                Hh = H_L[l]
                W = W_L[l]
                CW = W // Hh
                att_l = att_t[l]

                for tt0, sz in CHUNKS:
                    for gi, g in _chunk_groups(tt0, sz):
                        info = ginfo[gi]
                        lo_list, hi_list = info["lo"], info["hi"]
                        nch_lo, nch_hi = len(lo_list), len(hi_list)
                        nch2 = nch_lo + nch_hi
                        col0 = info["oo"]

                        ilo = smpool.tile([P, GROUP * MCH * 8], i16, tag="ilo")
                        nc.sync.dma_start(
                            out=ilo[:, :nch_lo * 8],
                            in_=inp["xlidxlo"][
                                :, info["io_lo"]:info["io_lo"] + nch_lo * 8])
                        ihi = smpool.tile([P, GROUP * MCH * 8], i16, tag="ihi")
                        nc.sync.dma_start(
                            out=ihi[:, :nch_hi * 8],
                            in_=inp["xlidxhi"][
                                :, info["io_hi"]:info["io_hi"] + nch_hi * 8])
                        oh_t = epool.tile([P, 2 * GROUP * MCH, P], f8,
                                          tag="oh")
                        nc.sync.dma_start(
                            out=oh_t[:, :nch2, :],
                            in_=inp["oh"][:, col0:col0 + nch2, :])
                        ohT_t = epool.tile([P, 2 * GROUP * MCH, P], f8,
                                           tag="ohT")
                        nc.sync.dma_start(
                            out=ohT_t[:, :nch2, :],
                            in_=inp["ohT"][:, col0:col0 + nch2, :])

                        # gather calls (<= GMAX idxs each) per stream;
                        # GMAX must stay <= half the SWDGE ring (2048 descs)
                        GMAX = 1024
                        def gathers(st, in_ap, idx_t, nch_s):
                            k = 0
                            while k < nch_s * P:
                                n = min(GMAX, nch_s * P - k)
                                nc.gpsimd.dma_gather(
                                    out_ap=st[:, k // P:(k + n) // P, :],
                                    in_ap=in_ap,
                                    idxs_ap=idx_t[:, k // 16:(k + n) // 16],
                                    num_idxs=n, num_idxs_reg=n, elem_size=P)
                                k += n
                        in_lo = (xl0lo[:, :] if l == 0
                                 else xl_full[l][0:HALF, :])
                        in_hi = (xl0hi[:, :] if l == 0
                                 else xl_full[l][HALF:NP_, :])
                        st_lo = epool.tile([P, GROUP * MCH, P], f16,
                                           tag="xlglo")
                        gathers(st_lo, in_lo, ilo, nch_lo)
                        st_hi = epool.tile([P, GROUP * MCH, P], f16,
                                           tag="xlghi")
                        gathers(st_hi, in_hi, ihi, nch_hi)

                        # u = xr[tloc] + xl_src  (PSUM), leaky-relu -> L
                        L_t = epool.tile([P, 2 * GROUP * MCH, P], f16, tag="L")
                        for c0, xt, clist in ((0, st_lo, lo_list),
                                              (nch_lo, st_hi, hi_list)):
                            for b0 in range(0, len(clist), BATCH):
                                nb = min(BATCH, len(clist) - b0)
                                psu = psU.tile([P, BATCH, P], f32,
                                               space="PSUM", tag="psu")
                                for k in range(nb):
                                    cc = b0 + k
                                    nc.tensor.matmul(
                                        out=psu[:, k, :W],
                                        lhsT=ohT_t[:, c0 + cc, :],
                                        rhs=xr_sb[:, clist[cc], :W],
                                        start=True, stop=False)
                                    nc.tensor.matmul(
                                        out=psu[:, k, :W], lhsT=ident_t[:],
                                        rhs=xt[:, cc, :W],
                                        start=False, stop=True)
                                nc.scalar.activation(
                                    out=L_t[:, c0 + b0:c0 + b0 + nb, :W],
                                    in_=psu[:, :nb, :W],
                                    func=mybir.ActivationFunctionType.Prelu,
                                    alpha=NEG)

                        # scores: L *= att ; tree-reduce over cw
                        nc.vector.tensor_tensor(
                            out=L_t[:, :nch2, :W], in0=L_t[:, :nch2, :W],
                            in1=att_l[:, :W].unsqueeze(1).broadcast_to(
                                [P, nch2, W]),
                            op=mybir.AluOpType.mult)
                        w_t = epool.tile([P, 2 * GROUP * MCH, P + 4], f16,
                                         tag="w")
                        Lv = L_t[:, :nch2, :W].rearrange(
                            "p c (w h) -> p c w h", h=Hh)
                        # tree halves in place into L_t's low columns (L is
                        # dead after the att-mult; keeps w_t alias-free)
                        tv = L_t[:, :nch2, :W // 2].rearrange(
                            "p c (w h) -> p c w h", h=Hh)
                        half = CW // 2
                        nc.vector.tensor_tensor(
                            out=tv[:, :, :half, :], in0=Lv[:, :, :half, :],
                            in1=Lv[:, :, half:, :], op=mybir.AluOpType.add)
                        while half > 1:
                            q = half // 2
                            nc.vector.tensor_tensor(
                                out=tv[:, :, :q, :], in0=tv[:, :, :q, :],
                                in1=tv[:, :, q:half, :], op=mybir.AluOpType.add)
                            half = q
                        # w values and alpha
                        nc.scalar.activation(
                            out=w_t[:, :nch2, W:W + Hh], in_=tv[:, :nch2, 0, :],
                            func=mybir.ActivationFunctionType.Exp)
                        a_b = w_t[:, :nch2, W:W + Hh].unsqueeze(2).broadcast_to(
                            [P, nch2, CW, Hh])
                        for c0, xt, clist in ((0, st_lo, lo_list),
                                              (nch_lo, st_hi, hi_list)):
                            ns = len(clist)
                            nc.vector.tensor_tensor(
                                out=w_t[:, c0:c0 + ns, :W].rearrange(
                                    "p c (w h) -> p c w h", h=Hh),
                                in0=xt[:, :ns, :W].rearrange(
                                    "p c (w h) -> p c w h", h=Hh),
                                in1=a_b[:, c0:c0 + ns],
                                op=mybir.AluOpType.mult)

                        if dbg and l == 0 and gi == 0:
                            nc.sync.dma_start(out=dbg["st0"][:, :nch_lo, :],
                                              in_=st_lo[:, :nch_lo, :])
                            nc.sync.dma_start(out=dbg["L0"][:, :nch2, :],
                                              in_=L_t[:, :nch2, :])
                            nc.sync.dma_start(out=dbg["w0"][:, :nch2, :],
                                              in_=w_t[:, :nch2, :])

                        # scatter per tile
                        for tt_ in range(g):
                            t = gi + tt_
                            cids = ([c0 for c0, tt in enumerate(lo_list)
                                     if tt == t]
                                    + [nch_lo + c0
                                       for c0, tt in enumerate(hi_list)
                                       if tt == t])
                            ps = psS.tile([P, P + 4], f32, space="PSUM",
                                          tag="pss")
                            for cix, cid in enumerate(cids):
                                nc.tensor.matmul(
                                    out=ps[:, :W + Hh],
                                    lhsT=oh_t[:, cid, :],
                                    rhs=w_t[:, cid, :W + Hh],
                                    start=(cix == 0),
                                    stop=(cix == len(cids) - 1))
                            rec = smpool.tile([P, 4], f32, tag="rec")
                            nc.vector.reciprocal(out=rec[:, :Hh],
                                                 in_=ps[:, W:W + Hh])
                            t1 = smpool.tile([P, P], f16, tag="t1")
                            nc.vector.tensor_tensor(
                                out=t1[:, :W].rearrange(
                                    "p (w h) -> p w h", h=Hh),
                                in0=ps[:, :W].rearrange(
                                    "p (w h) -> p w h", h=Hh),
                                in1=rec[:, :Hh].unsqueeze(1).broadcast_to(
                                    [P, CW, Hh]),
                                op=mybir.AluOpType.mult)
                            if l < 2:
                                nc.scalar.activation(
                                    out=stg_xn[:, t, :], in_=t1[:],
                                    func=mybir.ActivationFunctionType.Prelu,
                                    alpha=NEG)
                            else:
                                xnm = smpool.tile([P, G_GRAPHS], f16,
                                                  tag="xnm2")
                                nc.scalar.activation(
                                    out=xnm[:], in_=t1[:, :G_GRAPHS],
                                    func=mybir.ActivationFunctionType.Prelu,
                                    alpha=NEG)
                                nc.tensor.matmul(
                                    out=pool_psum[:],
                                    lhsT=pool_t[:, t, :], rhs=xnm[:],
                                    start=(t == 0), stop=(t == NTC - 1))

                    # ---- interleaved phase A(l+1) + chunk AllGather ----
                    if l < 2:
                        Wn = W_L[l + 1]
                        nc.sync.dma_start(
                            out=xn_own[tt0 * P:(tt0 + sz) * P, :].rearrange(
                                "(t p) f -> p t f", p=P),
                            in_=stg_xn[:, tt0:tt0 + sz, :])
                        for j0 in range(tt0, tt0 + sz, 2):
                            w_ = min(2, tt0 + sz - j0)
                            xs_t = stpool.tile([P, 2 * P], f16, tag="xstrip2")
                            nc.sync.dma_start_transpose(
                                out=xs_t[:, :w_ * P],
                                in_=xn_own[j0 * P:(j0 + w_) * P, :])
                            ps = psU.tile([P, SA, P], f32, space="PSUM",
                                          tag="psu")
                            for j in range(w_):
                                nc.tensor.matmul(
                                    out=ps[:, j, :Wn],
                                    lhsT=xs_t[:, j * P:(j + 1) * P],
                                    rhs=wlr_t[l + 1][:, :Wn],
                                    start=True, stop=True)
                                nc.tensor.matmul(
                                    out=ps[:, 4 + j, :Wn],
                                    lhsT=xs_t[:, j * P:(j + 1) * P],
                                    rhs=wlr_t[l + 1][:, 128:128 + Wn],
                                    start=True, stop=True)
                            psum_copy(stg_xl[:, j0:j0 + w_, :Wn],
                                      ps[:, :w_, :Wn])
                            psum_copy(xr_sb[:, j0:j0 + w_, :Wn],
                                      ps[:, 4:4 + w_, :Wn])
                        Ws = xl_own[l + 1].shape[1]
                        nc.sync.dma_start(
                            out=xl_own[l + 1][tt0 * P:(tt0 + sz) * P, :]
                            .rearrange("(t p) f -> p t f", p=P),
                            in_=stg_xl[:, tt0:tt0 + sz, :Ws])
                        s0 = CORES * tt0 * P
                        s1 = CORES * (tt0 + sz) * P
                        if Wn == P or os.environ.get("GAT_L2FULL"):
                            nc.gpsimd.collective_compute(
                                "AllGather", mybir.AluOpType.bypass,
                                replica_groups=[list(range(CORES))],
                                ins=[xl_own[l + 1][tt0 * P:(tt0 + sz) * P, :]],
                                outs=[xl_full[l + 1][s0:s1, :]])
                        else:
                            nc.gpsimd.collective_compute(
                                "AllGather", mybir.AluOpType.bypass,
                                replica_groups=[list(range(CORES))],
                                ins=[xl_own[l + 1][tt0 * P:(tt0 + sz) * P, :]],
                                outs=[xl2c[s0:s1, :]])
                            nc.sync.dma_start(
                                out=xl_full[l + 1][s0:s1, :Wn],
                                in_=xl2c[s0:s1, :])

                if dbg and l == 0:
                    nc.sync.dma_start(out=dbg["xl0"][0:HALF, :], in_=xl0lo[:])
                    nc.sync.dma_start(out=dbg["xl0"][HALF:NP_, :],
                                      in_=xl0hi[:])
                    nc.sync.dma_start(out=dbg["xn0"][:], in_=xn_own[:])
                if dbg and l == 1:
                    nc.sync.dma_start(out=dbg["xl1"][:], in_=xl_full[1][:])

            pool_sb = smpool.tile([G_GRAPHS, G_GRAPHS], f32, tag="poolsb")
            nc.vector.tensor_copy(out=pool_sb[:], in_=pool_psum[:])
            nc.sync.dma_start(out=pooled[:], in_=pool_sb[:])

    nc.finalize()
    return nc


def kernel(**inputs):
    x = np.asarray(inputs["x"])
    edge_index = np.asarray(inputs["edge_index"])
    batch = np.asarray(inputs["batch"])
    params = []
    for l in range(3):
        params.append((np.asarray(inputs[f"Wl{l}"]),
                       np.asarray(inputs[f"Wr{l}"]),
                       np.asarray(inputs[f"att{l}"])))
        b = np.asarray(inputs[f"b{l}"])
        assert np.all(b == 0), "nonzero bias not supported"

    meta, in_maps = _preprocess(x, edge_index, batch, params)

    key = ("nc", meta["chs_lo"], meta["chs_hi"])
    if key not in _CACHE:
        _CACHE[key] = _build(meta)
    nc = _CACHE[key]

    try:
        res = run_bass_kernel_spmd(
            nc, in_maps, core_ids=list(range(CORES)),
            trace=bool(os.environ.get("GAT_TRACE")))
    except ModuleNotFoundError:
        res = run_bass_kernel_spmd(nc, in_maps, core_ids=list(range(CORES)))
    kernel._last_result = res

    pooled = np.zeros((G_GRAPHS, G_GRAPHS), np.float64)
    for c in range(CORES):
        pooled += res.results[c]["pooled"].astype(np.float64)
    cnt = np.bincount(batch, minlength=G_GRAPHS).astype(np.float64)
    out = pooled / np.maximum(cnt, 1.0)[:, None]
    return out.astype(np.float32)


# revision 70
# speedup vs baseline: 1.0713x; 1.0063x over previous
"""GATv2 (3-layer, heads=4/4/1) full-graph kernel for 8 Trainium2 NeuronCores.

Contract: kernel(**inputs) takes the FULL unsharded inputs (as produced by
setup_inputs()) and returns the FULL [64, 64] float32 output.

v4 design (vs. v3 baseline at 1.90 ms):
- Layer-0 phase A is REPLICATED: every core computes xl0 for all 50176 nodes
  straight from the (fully available) input x and writes it to local DRAM.
  The 251 us layer-0 feature AllGather is gone entirely.
- Layers 1-2 exchange xl via CHUNKED AllGathers overlapped with compute:
  phase A(l+1) for a chunk of own tiles runs as soon as phase B(l) finishes
  those tiles, and the chunk's AllGather fires immediately, running on the
  collective cores while phase B(l) continues on later chunks.  xl_full rows
  are laid out (chunk, core, row)-major so every AllGather lands in a
  contiguous slice with IR identical on all cores.
- Per-edge xl gathers run as 1024-index dma_gather calls (half the
  2048-descriptor SWDGE ring, so two stay in flight); nodes are relabeled by
  a degree-balancing permutation and chunk counts are sized per tile slot
  (max over cores) instead of one global worst case.
- Pad targets (node ids >= N) get a fake self-loop so every target has a
  nonzero softmax denominator; per-tile normalization is then a single DVE
  divide (no max/reciprocal dance, no NaNs reaching the pool matmul).
- Per-edge xr is expanded on the PE from SBUF-resident xr tiles via host-built
  fp8 one-hot matrices (oh: [lane,tgt], ohT: [tgt,lane]); gathered xl rows are
  accumulated into the same PSUM via an identity matmul; leaky-relu applied
  straight from PSUM on ACT; scores via DVE mult + halving-tree; softmax
  without max-shift (scores empirically in [-8, 7]); scatter-sum + denominators
  via fp8 one-hot matmul into PSUM; global-mean-pool partials via PE, summed
  and divided on the host.
"""
import os
import numpy as np
import ml_dtypes

import concourse.bacc as bacc
import concourse.mybir as mybir
import concourse.tile as tile
from concourse._compat import get_trn_type
from concourse.bass_utils import run_bass_kernel_spmd

f16 = mybir.dt.float16
f32 = mybir.dt.float32
f8 = mybir.dt.float8e4
i16 = mybir.dt.int16
f8np = ml_dtypes.float8_e4m3

P = 128
N = 50000
E = 800000
NP_ = 50176            # padded nodes = 392 * 128
NT = NP_ // P          # 392 global tiles
CORES = 8
NTC = NT // CORES      # 49 tiles per core
NC_NODES = NTC * P     # 6272 nodes per core
HALF = NP_ // 2        # 25088 rows per shared half
G_GRAPHS = 64
NEG = 0.2
GROUP = 2              # tiles per gather/compute group
BATCH = 8              # chunks per PSUM u-batch
H_L = [4, 4, 1]
W_L = [128, 128, 64]   # xl/value width per layer
CHUNKS = [(0, 13), (13, 12), (25, 12), (37, 12)]  # (tt0, sz) ag-pipeline chunks

_CACHE = {}


def _pack_idx_image(seq):
    """int16 index sequence -> gather SBUF image [128, len/16]."""
    n = len(seq)
    assert n % 16 == 0
    img = np.asarray(seq, np.int16).reshape(n // 16, 16).T
    return np.tile(img, (8, 1))


def _chunk_groups(tt0, sz):
    out = []
    gi = tt0
    while gi < tt0 + sz:
        out.append((gi, min(GROUP, tt0 + sz - gi)))
        gi += GROUP
    return out


def _group_list():
    out = []
    for tt0, sz in CHUNKS:
        out.extend(_chunk_groups(tt0, sz))
    return out


def _tile_slot():
    """global tile t -> slot in the (chunk, core, row) xl_full layout."""
    slot = np.empty(NT, np.int64)
    for c in range(CORES):
        for tt0, sz in CHUNKS:
            for i in range(sz):
                slot[NTC * c + tt0 + i] = CORES * tt0 + c * sz + i
    return slot


def _pack_perm(h, c):
    """column permutation: packed[cw*h_n + hh] = natural[hh*c + cw]."""
    perm = np.empty(h * c, np.int64)
    for cw in range(c):
        for hh in range(h):
            perm[cw * h + hh] = hh * c + cw
    return perm


def _balance_perm(edge_index):
    """Relabel real nodes so every 128-node tile has near-equal in-degree.
    Returns perm[orig] -> new position (pads N..NP_ stay in place)."""
    import heapq
    deg = np.bincount(edge_index[1].astype(np.int64), minlength=N) + 1
    order = np.argsort(-deg, kind="stable")
    nfull = N // P                      # 390 full tiles
    caps = [P] * nfull + [N - nfull * P]  # tile 390 gets the remainder
    heap = [(0, b) for b in range(len(caps))]
    heapq.heapify(heap)
    fill = [0] * len(caps)
    perm = np.empty(N, np.int64)
    for v in order:
        while True:
            s, b = heapq.heappop(heap)
            if fill[b] < caps[b]:
                break
        perm[v] = b * P + fill[b]
        fill[b] += 1
        if fill[b] < caps[b]:
            heapq.heappush(heap, (s + int(deg[v]), b))
    return perm


def _preprocess(x, edge_index, batch, params):
    nperm = _balance_perm(edge_index)
    loops = np.arange(N, dtype=np.int64)
    pads = np.arange(N, NP_, dtype=np.int64)   # fake self-loops on pad targets
    src = np.concatenate([nperm[edge_index[0].astype(np.int64)], nperm[loops],
                          pads])
    tgt = np.concatenate([nperm[edge_index[1].astype(np.int64)], nperm[loops],
                          pads])
    order = np.argsort(tgt, kind="stable")
    srcs, tgts = src[order], tgt[order]

    slot = _tile_slot()
    src_row = slot[srcs // P] * P + srcs % P   # permuted xl_full row per edge
    islo = src_row < HALF

    bounds = np.searchsorted(tgts, np.arange(0, NP_ + 1, P))
    nlo = np.empty(NT, np.int64)
    nhi = np.empty(NT, np.int64)
    for t in range(NT):
        s, e = bounds[t], bounds[t + 1]
        nlo[t] = int(islo[s:e].sum())
        nhi[t] = (e - s) - nlo[t]
    # per-core-tile-slot chunk counts (max over cores, static across SPMD IR)
    chs_lo = tuple(
        int(max(1, -(-nlo[tt::NTC].max() // P))) for tt in range(NTC))
    chs_hi = tuple(
        int(max(1, -(-nhi[tt::NTC].max() // P))) for tt in range(NTC))

    # per-layer packed weights / attention
    wlrs, att_reps = [], []
    prev_perm = None  # input-feature permutation (packing of previous layer)
    for li, (Wl, Wr, att) in enumerate(params):
        h, c = att.shape
        hc = h * c
        Wl = np.asarray(Wl, np.float32)
        Wr = np.asarray(Wr, np.float32)
        if prev_perm is not None:
            Wl = Wl[prev_perm]
            Wr = Wr[prev_perm]
        if li < 2:
            perm = _pack_perm(h, c)
            Wl = Wl[:, perm]
            Wr = Wr[:, perm]
            att_flat = np.asarray(att, np.float32).reshape(-1)[perm]
            prev_perm = perm
        else:
            att_flat = np.asarray(att, np.float32).reshape(-1)
            prev_perm = None
        wlr = np.zeros((P, 256), np.float16)
        wlr[: Wl.shape[0], :hc] = Wl.astype(np.float16)
        wlr[: Wr.shape[0], 128 : 128 + hc] = Wr.astype(np.float16)
        wlrs.append(wlr)
        af = np.zeros(P, np.float16)
        af[:hc] = att_flat.astype(np.float16)
        att_reps.append(np.tile(af[None, :], (P, 1)))

    ident = np.eye(P).astype(f8np)

    x_pad = np.zeros((NP_, P), np.float32)
    x_pad[nperm] = np.asarray(x, np.float32)   # rows at balanced positions
    # x columns in slot-major (permuted) order, shared by all cores
    slot_tile = np.empty(NT, np.int64)
    slot_tile[slot] = np.arange(NT)
    xTp = np.empty((P, NP_), np.float16)
    for s in range(NT):
        t = slot_tile[s]
        xTp[:, s * P:(s + 1) * P] = x_pad[t * P:(t + 1) * P].astype(np.float16).T

    grp_list = _group_list()

    # graph id per NEW position (pads -> 0, masked out by valid)
    batch_perm = np.zeros(NP_, np.int64)
    batch_perm[nperm] = np.asarray(batch, np.int64)
    valid_perm = np.zeros(NP_, bool)
    valid_perm[nperm] = True

    in_maps = []
    for c in range(CORES):
        t0 = c * NTC
        base = t0 * P
        # per-(tile, stream) slot tables, padded to chs_*[tt]*128
        xlo = [np.zeros(chs_lo[tt] * P, np.int64) for tt in range(NTC)]
        xhi = [np.zeros(chs_hi[tt] * P, np.int64) for tt in range(NTC)]
        tl_lo = [np.full(chs_lo[tt] * P, -1, np.int64) for tt in range(NTC)]
        tl_hi = [np.full(chs_hi[tt] * P, -1, np.int64) for tt in range(NTC)]
        for tt in range(NTC):
            t = t0 + tt
            s, e = bounds[t], bounds[t + 1]
            sl = tgts[s:e] - t * P
            sp = src_row[s:e]
            lo_mask = islo[s:e]
            k = int(lo_mask.sum()); k2 = (e - s) - k
            xlo[tt][:k] = sp[lo_mask]
            tl_lo[tt][:k] = sl[lo_mask]
            xhi[tt][:k2] = sp[~lo_mask] - HALF
            tl_hi[tt][:k2] = sl[~lo_mask]

        # group-stream-major chunk columns
        lo_imgs, hi_imgs = [], []
        oh_cols, ohT_cols = [], []
        for gi, g in grp_list:
            lo_seq = np.concatenate(xlo[gi:gi + g])
            hi_seq = np.concatenate(xhi[gi:gi + g])
            lo_imgs.append(_pack_idx_image(lo_seq))
            hi_imgs.append(_pack_idx_image(hi_seq))
            tl_seq = np.concatenate(tl_lo[gi:gi + g] + tl_hi[gi:gi + g])
            nch2 = len(tl_seq) // P
            tl_mat = tl_seq.reshape(nch2, P)          # [chunk, lane] -> tloc
            oh = np.zeros((P, nch2, P), f8np)         # [lane, chunk, tgt]
            ohT = np.zeros((P, nch2, P), f8np)        # [tgt, chunk, lane]
            ch_i, ln_i = np.nonzero(tl_mat >= 0)
            tl_v = tl_mat[ch_i, ln_i]
            oh[ln_i, ch_i, tl_v] = 1.0
            ohT[tl_v, ch_i, ln_i] = 1.0
            oh_cols.append(oh)
            ohT_cols.append(ohT)

        # pooling one-hot [128, NTC, 64]
        pool = np.zeros((P, NTC, G_GRAPHS), np.float16)
        for tt in range(NTC):
            gn = base + tt * P + np.arange(P)
            valid = valid_perm[gn]
            pool[valid, tt, batch_perm[gn[valid]]] = 1.0

        # own-shard x columns (natural tt order) for the layer-0 xr pass
        xr0T = np.ascontiguousarray(
            x_pad[base:base + NC_NODES].astype(np.float16).T)

        in_maps.append({
            "x0T": xTp,
            "xr0T": xr0T,
            "xlidxlo": np.concatenate(lo_imgs, axis=1),
            "xlidxhi": np.concatenate(hi_imgs, axis=1),
            "oh": np.concatenate(oh_cols, axis=1),
            "ohT": np.concatenate(ohT_cols, axis=1),
            "ident": ident,
            "attr0": att_reps[0], "attr1": att_reps[1], "attr2": att_reps[2],
            "wlr0": wlrs[0], "wlr1": wlrs[1], "wlr2": wlrs[2],
            "pooloh": pool,
        })

    return dict(chs_lo=chs_lo, chs_hi=chs_hi), in_maps


def _build(meta):
    chs_lo, chs_hi = meta["chs_lo"], meta["chs_hi"]
    NIL = sum(chs_lo)   # lo chunks per core
    NIH = sum(chs_hi)
    NCH = NIL + NIH     # total chunk columns per core
    MCH = max(max(chs_lo), max(chs_hi))
    nc = bacc.Bacc(
        get_trn_type() or "TRN2",
        target_bir_lowering=False,
        debug=False,
        num_devices=CORES,
        dynamic_dma_scratch_size=32768,   # 2048-descriptor SWDGE ring
    )
    inp = {}
    for name, shape, dt in [
        ("x0T", [P, NP_], f16),
        ("xr0T", [P, NC_NODES], f16),
        ("xlidxlo", [P, NIL * 8], i16),
        ("xlidxhi", [P, NIH * 8], i16),
        ("oh", [P, NCH, P], f8),
        ("ohT", [P, NCH, P], f8),
        ("ident", [P, P], f8),
        ("attr0", [P, P], f16), ("attr1", [P, P], f16), ("attr2", [P, P], f16),
        ("wlr0", [P, 256], f16), ("wlr1", [P, 256], f16), ("wlr2", [P, 256], f16),
        ("pooloh", [P, NTC, G_GRAPHS], f16),
    ]:
        inp[name] = nc.dram_tensor(name, shape, dt, kind="ExternalInput")

    pooled = nc.dram_tensor("pooled", [G_GRAPHS, G_GRAPHS], f32,
                            kind="ExternalOutput")
    dbg = {}
    if os.environ.get("GAT_DEBUG"):
        dbg["xl0"] = nc.dram_tensor("dbg_xl0", [NP_, P], f16,
                                    kind="ExternalOutput")
        dbg["xn0"] = nc.dram_tensor("dbg_xn0", [NC_NODES, P], f16,
                                    kind="ExternalOutput")
        dbg["xl1"] = nc.dram_tensor("dbg_xl1", [NP_, P], f16,
                                    kind="ExternalOutput")
        dbg["st0"] = nc.dram_tensor("dbg_st0", [P, 64, P], f16,
                                    kind="ExternalOutput")
        dbg["L0"] = nc.dram_tensor("dbg_L0", [P, 64, P], f16,
                                   kind="ExternalOutput")
        dbg["w0"] = nc.dram_tensor("dbg_w0", [P, 64, P + 4], f16,
                                   kind="ExternalOutput")

    # xl_full[l]: per-edge gather source, rows in (chunk, core, row) slot order
    xl_full = [
        nc.dram_tensor("xl_full0", [NP_, P], f16),
        nc.dram_tensor("xl_full1", [NP_, P], f16, addr_space="Shared"),
        nc.dram_tensor("xl_full2", [NP_, P], f16),
    ]
    # layer-2 ag payload is only 64 wide; gathered rows must still be 256B,
    # so ag lands compact and a local DMA expands into xl_full2's row pitch
    xl2c = nc.dram_tensor("xl_full2c", [NP_, W_L[2]], f16, addr_space="Shared")
    w_own2 = P if os.environ.get("GAT_L2FULL") else W_L[2]
    xl_own = [
        None,
        nc.dram_tensor("xl_own1", [NC_NODES, P], f16),
        nc.dram_tensor("xl_own2", [NC_NODES, w_own2], f16),
    ]
    xn_own = nc.dram_tensor("xn_own", [NC_NODES, P], f16)

    grp_list = _group_list()
    # per-group descriptors: chunk lists per stream, cumulative offsets
    ginfo = {}
    a_lo = a_hi = a_o = 0
    for gi, g in grp_list:
        lo_list = [tt for tt in range(gi, gi + g) for _ in range(chs_lo[tt])]
        hi_list = [tt for tt in range(gi, gi + g) for _ in range(chs_hi[tt])]
        ginfo[gi] = dict(lo=lo_list, hi=hi_list, io_lo=a_lo, io_hi=a_hi,
                         oo=a_o)
        a_lo += len(lo_list) * 8
        a_hi += len(hi_list) * 8
        a_o += len(lo_list) + len(hi_list)

    SA = 8  # tiles per phase-A strip (shares the [P,8,128] psu PSUM tag)

    with tile.TileContext(nc) as tc:
        with (
            tc.tile_pool(name="const", bufs=1) as cpool,
            tc.tile_pool(name="stage", bufs=1) as spool,
            tc.tile_pool(name="strip", bufs=3) as stpool,
            tc.tile_pool(name="edge", bufs=3) as epool,
            tc.tile_pool(name="small", bufs=6) as smpool,
            tc.tile_pool(name="psU", bufs=2, space="PSUM") as psU,
            tc.tile_pool(name="psS", bufs=3, space="PSUM") as psS,
            tc.tile_pool(name="psP", bufs=1, space="PSUM") as psP,
        ):
            ident_t = cpool.tile([P, P], f8)
            nc.sync.dma_start(out=ident_t[:], in_=inp["ident"][:])
            pool_t = cpool.tile([P, NTC, G_GRAPHS], f16)
            nc.sync.dma_start(out=pool_t[:], in_=inp["pooloh"][:])
            wlr_t, att_t = [], []
            for l in range(3):
                w_t_ = cpool.tile([P, 256], f16, tag=f"wlr{l}")
                nc.sync.dma_start(out=w_t_[:], in_=inp[f"wlr{l}"][:])
                wlr_t.append(w_t_)
                a_t_ = cpool.tile([P, P], f16, tag=f"att{l}")
                nc.sync.dma_start(out=a_t_[:], in_=inp[f"attr{l}"][:])
                att_t.append(a_t_)

            pool_psum = psP.tile([G_GRAPHS, G_GRAPHS], f32, space="PSUM")

            # persistent per-layer state
            xr_sb = spool.tile([P, NTC, P], f16, tag="xr_sb")
            stg_xl = spool.tile([P, NTC, P], f16, tag="stg_xl")
            stg_xn = spool.tile([P, NTC, P], f16, tag="stg_xn")

            ncopy = [0]

            def psum_copy(dst, src):
                # alternate PSUM->SBUF copies between ACT and DVE
                if ncopy[0] % 2 == 0:
                    nc.scalar.copy(out=dst, in_=src)
                else:
                    nc.vector.tensor_copy(out=dst, in_=src)
                ncopy[0] += 1

            # ---- replicated phase A, layer 0: xl0 for ALL slots ----
            for s0 in range(0, NT, SA):
                w_ = min(SA, NT - s0)
                xs_t = stpool.tile([P, SA * P], f16, tag="xstrip")
                nc.sync.dma_start(out=xs_t[:, :w_ * P],
                                  in_=inp["x0T"][:, s0 * P:(s0 + w_) * P])
                ps = psU.tile([P, SA, P], f32, space="PSUM", tag="psu")
                for j in range(w_):
                    nc.tensor.matmul(
                        out=ps[:, j, :], lhsT=xs_t[:, j * P:(j + 1) * P],
                        rhs=wlr_t[0][:, :P], start=True, stop=True)
                stg = stpool.tile([P, SA, P], f16, tag="a0stg")
                psum_copy(stg[:, :w_, :], ps[:, :w_, :])
                nc.sync.dma_start(
                    out=xl_full[0][s0 * P:(s0 + w_) * P, :].rearrange(
                        "(t p) f -> p t f", p=P),
                    in_=stg[:, :w_, :])
            # layer-0 xr for own tiles
            for s0 in range(0, NTC, SA):
                w_ = min(SA, NTC - s0)
                xs_t = stpool.tile([P, SA * P], f16, tag="xstrip")
                nc.sync.dma_start(out=xs_t[:, :w_ * P],
                                  in_=inp["xr0T"][:, s0 * P:(s0 + w_) * P])
                ps = psU.tile([P, SA, P], f32, space="PSUM", tag="psu")
                for j in range(w_):
                    nc.tensor.matmul(
                        out=ps[:, j, :], lhsT=xs_t[:, j * P:(j + 1) * P],
                        rhs=wlr_t[0][:, 128:256], start=True, stop=True)
                psum_copy(xr_sb[:, s0:s0 + w_, :], ps[:, :w_, :])

            # ---- layers ----
            for l in range(3):
                Hh = H_L[l]
                W = W_L[l]
                CW = W // Hh
                att_l = att_t[l]

                for tt0, sz in CHUNKS:
                    for gi, g in _chunk_groups(tt0, sz):
                        info = ginfo[gi]
                        lo_list, hi_list = info["lo"], info["hi"]
                        nch_lo, nch_hi = len(lo_list), len(hi_list)
                        nch2 = nch_lo + nch_hi
                        col0 = info["oo"]

                        ilo = smpool.tile([P, GROUP * MCH * 8], i16, tag="ilo")
                        nc.sync.dma_start(
                            out=ilo[:, :nch_lo * 8],
                            in_=inp["xlidxlo"][
                                :, info["io_lo"]:info["io_lo"] + nch_lo * 8])
                        ihi = smpool.tile([P, GROUP * MCH * 8], i16, tag="ihi")
                        nc.sync.dma_start(
                            out=ihi[:, :nch_hi * 8],
                            in_=inp["xlidxhi"][
                                :, info["io_hi"]:info["io_hi"] + nch_hi * 8])
                        oh_t = epool.tile([P, 2 * GROUP * MCH, P], f8,
                                          tag="oh")
                        nc.sync.dma_start(
                            out=oh_t[:, :nch2, :],
                            in_=inp["oh"][:, col0:col0 + nch2, :])
                        ohT_t = epool.tile([P, 2 * GROUP * MCH, P], f8,
                                           tag="ohT")
                        nc.sync.dma_start(
                            out=ohT_t[:, :nch2, :],
                            in_=inp["ohT"][:, col0:col0 + nch2, :])

                        # gather calls (<= GMAX idxs each) per stream;
                        # GMAX must stay <= half the SWDGE ring (2048 descs)
                        GMAX = 1024
                        def gathers(st, in_ap, idx_t, nch_s):
                            k = 0
                            while k < nch_s * P:
                                n = min(GMAX, nch_s * P - k)
                                nc.gpsimd.dma_gather(
                                    out_ap=st[:, k // P:(k + n) // P, :],
                                    in_ap=in_ap,
                                    idxs_ap=idx_t[:, k // 16:(k + n) // 16],
                                    num_idxs=n, num_idxs_reg=n, elem_size=P)
                                k += n
                        st_lo = epool.tile([P, GROUP * MCH, P], f16,
                                           tag="xlglo")
                        gathers(st_lo, xl_full[l][0:HALF, :], ilo, nch_lo)
                        st_hi = epool.tile([P, GROUP * MCH, P], f16,
                                           tag="xlghi")
                        gathers(st_hi, xl_full[l][HALF:NP_, :], ihi, nch_hi)

                        # u = xr[tloc] + xl_src  (PSUM), leaky-relu -> L
                        L_t = epool.tile([P, 2 * GROUP * MCH, P], f16, tag="L")
                        for c0, xt, clist in ((0, st_lo, lo_list),
                                              (nch_lo, st_hi, hi_list)):
                            for b0 in range(0, len(clist), BATCH):
                                nb = min(BATCH, len(clist) - b0)
                                psu = psU.tile([P, BATCH, P], f32,
                                               space="PSUM", tag="psu")
                                for k in range(nb):
                                    cc = b0 + k
                                    nc.tensor.matmul(
                                        out=psu[:, k, :W],
                                        lhsT=ohT_t[:, c0 + cc, :],
                                        rhs=xr_sb[:, clist[cc], :W],
                                        start=True, stop=False)
                                    nc.tensor.matmul(
                                        out=psu[:, k, :W], lhsT=ident_t[:],
                                        rhs=xt[:, cc, :W],
                                        start=False, stop=True)
                                nc.scalar.activation(
                                    out=L_t[:, c0 + b0:c0 + b0 + nb, :W],
                                    in_=psu[:, :nb, :W],
                                    func=mybir.ActivationFunctionType.Prelu,
                                    alpha=NEG)

                        # scores: L *= att ; tree-reduce over cw
                        nc.vector.tensor_tensor(
                            out=L_t[:, :nch2, :W], in0=L_t[:, :nch2, :W],
                            in1=att_l[:, :W].unsqueeze(1).broadcast_to(
                                [P, nch2, W]),
                            op=mybir.AluOpType.mult)
                        w_t = epool.tile([P, 2 * GROUP * MCH, P + 4], f16,
                                         tag="w")
                        Lv = L_t[:, :nch2, :W].rearrange(
                            "p c (w h) -> p c w h", h=Hh)
                        # tree halves in place into L_t's low columns (L is
                        # dead after the att-mult; keeps w_t alias-free)
                        tv = L_t[:, :nch2, :W // 2].rearrange(
                            "p c (w h) -> p c w h", h=Hh)
                        half = CW // 2
                        nc.vector.tensor_tensor(
                            out=tv[:, :, :half, :], in0=Lv[:, :, :half, :],
                            in1=Lv[:, :, half:, :], op=mybir.AluOpType.add)
                        while half > 1:
                            q = half // 2
                            nc.vector.tensor_tensor(
                                out=tv[:, :, :q, :], in0=tv[:, :, :q, :],
                                in1=tv[:, :, q:half, :], op=mybir.AluOpType.add)
                            half = q
                        # w values and alpha
                        nc.scalar.activation(
                            out=w_t[:, :nch2, W:W + Hh], in_=tv[:, :nch2, 0, :],
                            func=mybir.ActivationFunctionType.Exp)
                        a_b = w_t[:, :nch2, W:W + Hh].unsqueeze(2).broadcast_to(
                            [P, nch2, CW, Hh])
                        for c0, xt, clist in ((0, st_lo, lo_list),
                                              (nch_lo, st_hi, hi_list)):
                            ns = len(clist)
                            nc.vector.tensor_tensor(
                                out=w_t[:, c0:c0 + ns, :W].rearrange(
                                    "p c (w h) -> p c w h", h=Hh),
                                in0=xt[:, :ns, :W].rearrange(
                                    "p c (w h) -> p c w h", h=Hh),
                                in1=a_b[:, c0:c0 + ns],
                                op=mybir.AluOpType.mult)

                        if dbg and l == 0 and gi == 0:
                            nc.sync.dma_start(out=dbg["st0"][:, :nch_lo, :],
                                              in_=st_lo[:, :nch_lo, :])
                            nc.sync.dma_start(out=dbg["L0"][:, :nch2, :],
                                              in_=L_t[:, :nch2, :])
                            nc.sync.dma_start(out=dbg["w0"][:, :nch2, :],
                                              in_=w_t[:, :nch2, :])

                        # scatter per tile
                        for tt_ in range(g):
                            t = gi + tt_
                            cids = ([c0 for c0, tt in enumerate(lo_list)
                                     if tt == t]
                                    + [nch_lo + c0
                                       for c0, tt in enumerate(hi_list)
                                       if tt == t])
                            ps = psS.tile([P, P + 4], f32, space="PSUM",
                                          tag="pss")
                            for cix, cid in enumerate(cids):
                                nc.tensor.matmul(
                                    out=ps[:, :W + Hh],
                                    lhsT=oh_t[:, cid, :],
                                    rhs=w_t[:, cid, :W + Hh],
                                    start=(cix == 0),
                                    stop=(cix == len(cids) - 1))
                            rec = smpool.tile([P, 4], f32, tag="rec")
                            nc.vector.reciprocal(out=rec[:, :Hh],
                                                 in_=ps[:, W:W + Hh])
                            t1 = smpool.tile([P, P], f16, tag="t1")
                            nc.vector.tensor_tensor(
                                out=t1[:, :W].rearrange(
                                    "p (w h) -> p w h", h=Hh),
                                in0=ps[:, :W].rearrange(
                                    "p (w h) -> p w h", h=Hh),
                                in1=rec[:, :Hh].unsqueeze(1).broadcast_to(
                                    [P, CW, Hh]),
                                op=mybir.AluOpType.mult)
                            if l < 2:
                                nc.scalar.activation(
                                    out=stg_xn[:, t, :], in_=t1[:],
                                    func=mybir.ActivationFunctionType.Prelu,
                                    alpha=NEG)
                            else:
                                xnm = smpool.tile([P, G_GRAPHS], f16,
                                                  tag="xnm2")
                                nc.scalar.activation(
                                    out=xnm[:], in_=t1[:, :G_GRAPHS],
                                    func=mybir.ActivationFunctionType.Prelu,
                                    alpha=NEG)
                                nc.tensor.matmul(
                                    out=pool_psum[:],
                                    lhsT=pool_t[:, t, :], rhs=xnm[:],
                                    start=(t == 0), stop=(t == NTC - 1))

                    # ---- interleaved phase A(l+1) + chunk AllGather ----
                    if l < 2:
                        Wn = W_L[l + 1]
                        nc.sync.dma_start(
                            out=xn_own[tt0 * P:(tt0 + sz) * P, :].rearrange(
                                "(t p) f -> p t f", p=P),
                            in_=stg_xn[:, tt0:tt0 + sz, :])
                        for j0 in range(tt0, tt0 + sz, 2):
                            w_ = min(2, tt0 + sz - j0)
                            xs_t = stpool.tile([P, 2 * P], f16, tag="xstrip2")
                            nc.sync.dma_start_transpose(
                                out=xs_t[:, :w_ * P],
                                in_=xn_own[j0 * P:(j0 + w_) * P, :])
                            ps = psU.tile([P, SA, P], f32, space="PSUM",
                                          tag="psu")
                            for j in range(w_):
                                nc.tensor.matmul(
                                    out=ps[:, j, :Wn],
                                    lhsT=xs_t[:, j * P:(j + 1) * P],
                                    rhs=wlr_t[l + 1][:, :Wn],
                                    start=True, stop=True)
                                nc.tensor.matmul(
                                    out=ps[:, 4 + j, :Wn],
                                    lhsT=xs_t[:, j * P:(j + 1) * P],
                                    rhs=wlr_t[l + 1][:, 128:128 + Wn],
                                    start=True, stop=True)
                            psum_copy(stg_xl[:, j0:j0 + w_, :Wn],
                                      ps[:, :w_, :Wn])
                            psum_copy(xr_sb[:, j0:j0 + w_, :Wn],
                                      ps[:, 4:4 + w_, :Wn])
                        Ws = xl_own[l + 1].shape[1]
                        nc.sync.dma_start(
                            out=xl_own[l + 1][tt0 * P:(tt0 + sz) * P, :]
                            .rearrange("(t p) f -> p t f", p=P),
                            in_=stg_xl[:, tt0:tt0 + sz, :Ws])
                        s0 = CORES * tt0 * P
                        s1 = CORES * (tt0 + sz) * P
                        if Wn == P or os.environ.get("GAT_L2FULL"):
                            nc.gpsimd.collective_compute(
                                "AllGather", mybir.AluOpType.bypass,
                                replica_groups=[list(range(CORES))],
                                ins=[xl_own[l + 1][tt0 * P:(tt0 + sz) * P, :]],
                                outs=[xl_full[l + 1][s0:s1, :]])
                        else:
                            nc.gpsimd.collective_compute(
                                "AllGather", mybir.AluOpType.bypass,
                                replica_groups=[list(range(CORES))],
                                ins=[xl_own[l + 1][tt0 * P:(tt0 + sz) * P, :]],
                                outs=[xl2c[s0:s1, :]])
                            nc.sync.dma_start(
                                out=xl_full[l + 1][s0:s1, :Wn],
                                in_=xl2c[s0:s1, :])

                if dbg and l == 0:
                    nc.sync.dma_start(out=dbg["xl0"][:], in_=xl_full[0][:])
                    nc.sync.dma_start(out=dbg["xn0"][:], in_=xn_own[:])
                if dbg and l == 1:
                    nc.sync.dma_start(out=dbg["xl1"][:], in_=xl_full[1][:])

            pool_sb = smpool.tile([G_GRAPHS, G_GRAPHS], f32, tag="poolsb")
            nc.vector.tensor_copy(out=pool_sb[:], in_=pool_psum[:])
            nc.sync.dma_start(out=pooled[:], in_=pool_sb[:])

    nc.finalize()
    return nc


def kernel(**inputs):
    x = np.asarray(inputs["x"])
    edge_index = np.asarray(inputs["edge_index"])
    batch = np.asarray(inputs["batch"])
    params = []
    for l in range(3):
        params.append((np.asarray(inputs[f"Wl{l}"]),
                       np.asarray(inputs[f"Wr{l}"]),
                       np.asarray(inputs[f"att{l}"])))
        b = np.asarray(inputs[f"b{l}"])
        assert np.all(b == 0), "nonzero bias not supported"

    meta, in_maps = _preprocess(x, edge_index, batch, params)

    key = ("nc", meta["chs_lo"], meta["chs_hi"])
    if key not in _CACHE:
        _CACHE[key] = _build(meta)
    nc = _CACHE[key]

    try:
        res = run_bass_kernel_spmd(
            nc, in_maps, core_ids=list(range(CORES)),
            trace=bool(os.environ.get("GAT_TRACE")))
    except ModuleNotFoundError:
        res = run_bass_kernel_spmd(nc, in_maps, core_ids=list(range(CORES)))
    kernel._last_result = res

    pooled = np.zeros((G_GRAPHS, G_GRAPHS), np.float64)
    for c in range(CORES):
        pooled += res.results[c]["pooled"].astype(np.float64)
    cnt = np.bincount(batch, minlength=G_GRAPHS).astype(np.float64)
    out = pooled / np.maximum(cnt, 1.0)[:, None]
    return out.astype(np.float32)


# revision 73
# speedup vs baseline: 1.0715x; 1.0002x over previous
"""GATv2 (3-layer, heads=4/4/1) full-graph kernel for 8 Trainium2 NeuronCores.

Contract: kernel(**inputs) takes the FULL unsharded inputs (as produced by
setup_inputs()) and returns the FULL [64, 64] float32 output.

v4 design (vs. v3 baseline at 1.90 ms):
- Layer-0 phase A is REPLICATED: every core computes xl0 for all 50176 nodes
  straight from the (fully available) input x and writes it to local DRAM.
  The 251 us layer-0 feature AllGather is gone entirely.
- Layers 1-2 exchange xl via CHUNKED AllGathers overlapped with compute:
  phase A(l+1) for a chunk of own tiles runs as soon as phase B(l) finishes
  those tiles, and the chunk's AllGather fires immediately, running on the
  collective cores while phase B(l) continues on later chunks.  xl_full rows
  are laid out (chunk, core, row)-major so every AllGather lands in a
  contiguous slice with IR identical on all cores.
- Per-edge xl gathers run as 1024-index dma_gather calls (half the
  2048-descriptor SWDGE ring, so two stay in flight); nodes are relabeled by
  a degree-balancing permutation and chunk counts are sized per tile slot
  (max over cores) instead of one global worst case.
- Pad targets (node ids >= N) get a fake self-loop so every target has a
  nonzero softmax denominator; per-tile normalization is reciprocal+multiply
  with no clamp (no NaNs can reach the pool matmul).
- Per-edge xr is expanded on the PE from SBUF-resident xr tiles via host-built
  fp8 one-hot matrices (oh: [lane,tgt], ohT: [tgt,lane]); gathered xl rows are
  accumulated into the same PSUM via an identity matmul; leaky-relu applied
  straight from PSUM on ACT; scores via DVE mult + halving-tree; softmax
  without max-shift (scores empirically in [-8, 7]); scatter-sum + denominators
  via fp8 one-hot matmul into PSUM; global-mean-pool partials via PE, summed
  and divided on the host.
"""
import os
import numpy as np
import ml_dtypes

import concourse.bacc as bacc
import concourse.mybir as mybir
import concourse.tile as tile
from concourse._compat import get_trn_type
from concourse.bass_utils import run_bass_kernel_spmd

f16 = mybir.dt.float16
f32 = mybir.dt.float32
f8 = mybir.dt.float8e4
i16 = mybir.dt.int16
f8np = ml_dtypes.float8_e4m3

P = 128
N = 50000
E = 800000
NP_ = 50176            # padded nodes = 392 * 128
NT = NP_ // P          # 392 global tiles
CORES = 8
NTC = NT // CORES      # 49 tiles per core
NC_NODES = NTC * P     # 6272 nodes per core
HALF = NP_ // 2        # 25088 rows per shared half
G_GRAPHS = 64
NEG = 0.2
GROUP = 2              # tiles per gather/compute group
BATCH = 8              # chunks per PSUM u-batch
H_L = [4, 4, 1]
W_L = [128, 128, 64]   # xl/value width per layer
CHUNKS = [(0, 13), (13, 12), (25, 12), (37, 12)]  # (tt0, sz) ag-pipeline chunks

_CACHE = {}


def _pack_idx_image(seq):
    """int16 index sequence -> gather SBUF image [128, len/16]."""
    n = len(seq)
    assert n % 16 == 0
    img = np.asarray(seq, np.int16).reshape(n // 16, 16).T
    return np.tile(img, (8, 1))


def _chunk_groups(tt0, sz):
    out = []
    gi = tt0
    while gi < tt0 + sz:
        out.append((gi, min(GROUP, tt0 + sz - gi)))
        gi += GROUP
    return out


def _group_list():
    out = []
    for tt0, sz in CHUNKS:
        out.extend(_chunk_groups(tt0, sz))
    return out


def _tile_slot():
    """global tile t -> slot in the (chunk, core, row) xl_full layout."""
    slot = np.empty(NT, np.int64)
    for c in range(CORES):
        for tt0, sz in CHUNKS:
            for i in range(sz):
                slot[NTC * c + tt0 + i] = CORES * tt0 + c * sz + i
    return slot


def _pack_perm(h, c):
    """column permutation: packed[cw*h_n + hh] = natural[hh*c + cw]."""
    perm = np.empty(h * c, np.int64)
    for cw in range(c):
        for hh in range(h):
            perm[cw * h + hh] = hh * c + cw
    return perm


def _balance_perm(edge_index):
    """Relabel real nodes so every 128-node tile has near-equal in-degree.
    Returns perm[orig] -> new position (pads N..NP_ stay in place)."""
    import heapq
    deg = np.bincount(edge_index[1].astype(np.int64), minlength=N) + 1
    order = np.argsort(-deg, kind="stable")
    nfull = N // P                      # 390 full tiles
    caps = [P] * nfull + [N - nfull * P]  # tile 390 gets the remainder
    heap = [(0, b) for b in range(len(caps))]
    heapq.heapify(heap)
    fill = [0] * len(caps)
    perm = np.empty(N, np.int64)
    for v in order:
        while True:
            s, b = heapq.heappop(heap)
            if fill[b] < caps[b]:
                break
        perm[v] = b * P + fill[b]
        fill[b] += 1
        if fill[b] < caps[b]:
            heapq.heappush(heap, (s + int(deg[v]), b))
    return perm


def _preprocess(x, edge_index, batch, params):
    nperm = _balance_perm(edge_index)
    loops = np.arange(N, dtype=np.int64)
    pads = np.arange(N, NP_, dtype=np.int64)   # fake self-loops on pad targets
    src = np.concatenate([nperm[edge_index[0].astype(np.int64)], nperm[loops],
                          pads])
    tgt = np.concatenate([nperm[edge_index[1].astype(np.int64)], nperm[loops],
                          pads])
    order = np.argsort(tgt, kind="stable")
    srcs, tgts = src[order], tgt[order]

    slot = _tile_slot()
    src_row = slot[srcs // P] * P + srcs % P   # permuted xl_full row per edge
    islo = src_row < HALF

    bounds = np.searchsorted(tgts, np.arange(0, NP_ + 1, P))
    nlo = np.empty(NT, np.int64)
    nhi = np.empty(NT, np.int64)
    for t in range(NT):
        s, e = bounds[t], bounds[t + 1]
        nlo[t] = int(islo[s:e].sum())
        nhi[t] = (e - s) - nlo[t]
    # per-core-tile-slot chunk counts (max over cores, static across SPMD IR)
    chs_lo = tuple(
        int(max(1, -(-nlo[tt::NTC].max() // P))) for tt in range(NTC))
    chs_hi = tuple(
        int(max(1, -(-nhi[tt::NTC].max() // P))) for tt in range(NTC))

    # per-layer packed weights / attention
    wlrs, att_reps = [], []
    prev_perm = None  # input-feature permutation (packing of previous layer)
    for li, (Wl, Wr, att) in enumerate(params):
        h, c = att.shape
        hc = h * c
        Wl = np.asarray(Wl, np.float32)
        Wr = np.asarray(Wr, np.float32)
        if prev_perm is not None:
            Wl = Wl[prev_perm]
            Wr = Wr[prev_perm]
        if li < 2:
            perm = _pack_perm(h, c)
            Wl = Wl[:, perm]
            Wr = Wr[:, perm]
            att_flat = np.asarray(att, np.float32).reshape(-1)[perm]
            prev_perm = perm
        else:
            att_flat = np.asarray(att, np.float32).reshape(-1)
            prev_perm = None
        wlr = np.zeros((P, 256), np.float16)
        wlr[: Wl.shape[0], :hc] = Wl.astype(np.float16)
        wlr[: Wr.shape[0], 128 : 128 + hc] = Wr.astype(np.float16)
        wlrs.append(wlr)
        af = np.zeros(P, np.float16)
        af[:hc] = att_flat.astype(np.float16)
        att_reps.append(np.tile(af[None, :], (P, 1)))

    ident = np.eye(P).astype(f8np)

    x_pad = np.zeros((NP_, P), np.float32)
    x_pad[nperm] = np.asarray(x, np.float32)   # rows at balanced positions
    # x columns in slot-major (permuted) order, shared by all cores
    slot_tile = np.empty(NT, np.int64)
    slot_tile[slot] = np.arange(NT)
    xTp = np.empty((P, NP_), np.float16)
    for s in range(NT):
        t = slot_tile[s]
        xTp[:, s * P:(s + 1) * P] = x_pad[t * P:(t + 1) * P].astype(np.float16).T

    grp_list = _group_list()

    # graph id per NEW position (pads -> 0, masked out by valid)
    batch_perm = np.zeros(NP_, np.int64)
    batch_perm[nperm] = np.asarray(batch, np.int64)
    valid_perm = np.zeros(NP_, bool)
    valid_perm[nperm] = True

    in_maps = []
    for c in range(CORES):
        t0 = c * NTC
        base = t0 * P
        # per-(tile, stream) slot tables, padded to chs_*[tt]*128
        xlo = [np.zeros(chs_lo[tt] * P, np.int64) for tt in range(NTC)]
        xhi = [np.zeros(chs_hi[tt] * P, np.int64) for tt in range(NTC)]
        tl_lo = [np.full(chs_lo[tt] * P, -1, np.int64) for tt in range(NTC)]
        tl_hi = [np.full(chs_hi[tt] * P, -1, np.int64) for tt in range(NTC)]
        for tt in range(NTC):
            t = t0 + tt
            s, e = bounds[t], bounds[t + 1]
            sl = tgts[s:e] - t * P
            sp = src_row[s:e]
            lo_mask = islo[s:e]
            k = int(lo_mask.sum()); k2 = (e - s) - k
            xlo[tt][:k] = sp[lo_mask]
            tl_lo[tt][:k] = sl[lo_mask]
            xhi[tt][:k2] = sp[~lo_mask] - HALF
            tl_hi[tt][:k2] = sl[~lo_mask]

        # group-stream-major chunk columns
        lo_imgs, hi_imgs = [], []
        oh_cols, ohT_cols = [], []
        for gi, g in grp_list:
            lo_seq = np.concatenate(xlo[gi:gi + g])
            hi_seq = np.concatenate(xhi[gi:gi + g])
            lo_imgs.append(_pack_idx_image(lo_seq))
            hi_imgs.append(_pack_idx_image(hi_seq))
            tl_seq = np.concatenate(tl_lo[gi:gi + g] + tl_hi[gi:gi + g])
            nch2 = len(tl_seq) // P
            tl_mat = tl_seq.reshape(nch2, P)          # [chunk, lane] -> tloc
            oh = np.zeros((P, nch2, P), f8np)         # [lane, chunk, tgt]
            ohT = np.zeros((P, nch2, P), f8np)        # [tgt, chunk, lane]
            ch_i, ln_i = np.nonzero(tl_mat >= 0)
            tl_v = tl_mat[ch_i, ln_i]
            oh[ln_i, ch_i, tl_v] = 1.0
            ohT[tl_v, ch_i, ln_i] = 1.0
            oh_cols.append(oh)
            ohT_cols.append(ohT)

        # pooling one-hot [128, NTC, 64]
        pool = np.zeros((P, NTC, G_GRAPHS), np.float16)
        for tt in range(NTC):
            gn = base + tt * P + np.arange(P)
            valid = valid_perm[gn]
            pool[valid, tt, batch_perm[gn[valid]]] = 1.0

        # own-shard x columns (natural tt order) for the layer-0 xr pass
        xr0T = np.ascontiguousarray(
            x_pad[base:base + NC_NODES].astype(np.float16).T)

        in_maps.append({
            "x0T": xTp,
            "xr0T": xr0T,
            "xlidxlo": np.concatenate(lo_imgs, axis=1),
            "xlidxhi": np.concatenate(hi_imgs, axis=1),
            "oh": np.concatenate(oh_cols, axis=1),
            "ohT": np.concatenate(ohT_cols, axis=1),
            "ident": ident,
            "attr0": att_reps[0], "attr1": att_reps[1], "attr2": att_reps[2],
            "wlr0": wlrs[0], "wlr1": wlrs[1], "wlr2": wlrs[2],
            "pooloh": pool,
        })

    return dict(chs_lo=chs_lo, chs_hi=chs_hi), in_maps


def _build(meta):
    chs_lo, chs_hi = meta["chs_lo"], meta["chs_hi"]
    NIL = sum(chs_lo)   # lo chunks per core
    NIH = sum(chs_hi)
    NCH = NIL + NIH     # total chunk columns per core
    MCH = max(max(chs_lo), max(chs_hi))
    nc = bacc.Bacc(
        get_trn_type() or "TRN2",
        target_bir_lowering=False,
        debug=False,
        num_devices=CORES,
        dynamic_dma_scratch_size=32768,   # 2048-descriptor SWDGE ring
    )
    inp = {}
    for name, shape, dt in [
        ("x0T", [P, NP_], f16),
        ("xr0T", [P, NC_NODES], f16),
        ("xlidxlo", [P, NIL * 8], i16),
        ("xlidxhi", [P, NIH * 8], i16),
        ("oh", [P, NCH, P], f8),
        ("ohT", [P, NCH, P], f8),
        ("ident", [P, P], f8),
        ("attr0", [P, P], f16), ("attr1", [P, P], f16), ("attr2", [P, P], f16),
        ("wlr0", [P, 256], f16), ("wlr1", [P, 256], f16), ("wlr2", [P, 256], f16),
        ("pooloh", [P, NTC, G_GRAPHS], f16),
    ]:
        inp[name] = nc.dram_tensor(name, shape, dt, kind="ExternalInput")

    pooled = nc.dram_tensor("pooled", [G_GRAPHS, G_GRAPHS], f32,
                            kind="ExternalOutput")
    dbg = {}
    if os.environ.get("GAT_DEBUG"):
        dbg["xl0"] = nc.dram_tensor("dbg_xl0", [NP_, P], f16,
                                    kind="ExternalOutput")
        dbg["xn0"] = nc.dram_tensor("dbg_xn0", [NC_NODES, P], f16,
                                    kind="ExternalOutput")
        dbg["xl1"] = nc.dram_tensor("dbg_xl1", [NP_, P], f16,
                                    kind="ExternalOutput")
        dbg["st0"] = nc.dram_tensor("dbg_st0", [P, 64, P], f16,
                                    kind="ExternalOutput")
        dbg["L0"] = nc.dram_tensor("dbg_L0", [P, 64, P], f16,
                                   kind="ExternalOutput")
        dbg["w0"] = nc.dram_tensor("dbg_w0", [P, 64, P + 4], f16,
                                   kind="ExternalOutput")

    # xl_full[l]: per-edge gather source, rows in (chunk, core, row) slot order
    xl_full = [
        nc.dram_tensor("xl_full0", [NP_, P], f16),
        nc.dram_tensor("xl_full1", [NP_, P], f16, addr_space="Shared"),
        nc.dram_tensor("xl_full2", [NP_, P], f16),
    ]
    # layer-2 ag payload is only 64 wide; gathered rows must still be 256B,
    # so ag lands compact and a local DMA expands into xl_full2's row pitch
    xl2c = nc.dram_tensor("xl_full2c", [NP_, W_L[2]], f16, addr_space="Shared")
    w_own2 = P if os.environ.get("GAT_L2FULL") else W_L[2]
    xl_own = [
        None,
        nc.dram_tensor("xl_own1", [NC_NODES, P], f16),
        nc.dram_tensor("xl_own2", [NC_NODES, w_own2], f16),
    ]
    xn_own = nc.dram_tensor("xn_own", [NC_NODES, P], f16)

    grp_list = _group_list()
    # per-group descriptors: chunk lists per stream, cumulative offsets
    ginfo = {}
    a_lo = a_hi = a_o = 0
    for gi, g in grp_list:
        lo_list = [tt for tt in range(gi, gi + g) for _ in range(chs_lo[tt])]
        hi_list = [tt for tt in range(gi, gi + g) for _ in range(chs_hi[tt])]
        ginfo[gi] = dict(lo=lo_list, hi=hi_list, io_lo=a_lo, io_hi=a_hi,
                         oo=a_o)
        a_lo += len(lo_list) * 8
        a_hi += len(hi_list) * 8
        a_o += len(lo_list) + len(hi_list)

    SA = 8  # tiles per phase-A strip (shares the [P,8,128] psu PSUM tag)

    with tile.TileContext(nc) as tc:
        with (
            tc.tile_pool(name="const", bufs=1) as cpool,
            tc.tile_pool(name="stage", bufs=1) as spool,
            tc.tile_pool(name="strip", bufs=3) as stpool,
            tc.tile_pool(name="edge", bufs=3) as epool,
            tc.tile_pool(name="small", bufs=8) as smpool,
            tc.tile_pool(name="psU", bufs=2, space="PSUM") as psU,
            tc.tile_pool(name="psS", bufs=3, space="PSUM") as psS,
            tc.tile_pool(name="psP", bufs=1, space="PSUM") as psP,
        ):
            ident_t = cpool.tile([P, P], f8)
            nc.sync.dma_start(out=ident_t[:], in_=inp["ident"][:])
            pool_t = cpool.tile([P, NTC, G_GRAPHS], f16)
            nc.sync.dma_start(out=pool_t[:], in_=inp["pooloh"][:])
            wlr_t, att_t = [], []
            for l in range(3):
                w_t_ = cpool.tile([P, 256], f16, tag=f"wlr{l}")
                nc.sync.dma_start(out=w_t_[:], in_=inp[f"wlr{l}"][:])
                wlr_t.append(w_t_)
                a_t_ = cpool.tile([P, P], f16, tag=f"att{l}")
                nc.sync.dma_start(out=a_t_[:], in_=inp[f"attr{l}"][:])
                att_t.append(a_t_)

            pool_psum = psP.tile([G_GRAPHS, G_GRAPHS], f32, space="PSUM")

            # persistent per-layer state
            xr_sb = spool.tile([P, NTC, P], f16, tag="xr_sb")
            stg_xl = spool.tile([P, NTC, P], f16, tag="stg_xl")
            stg_xn = spool.tile([P, NTC, P], f16, tag="stg_xn")

            ncopy = [0]

            def psum_copy(dst, src):
                # alternate PSUM->SBUF copies between ACT and DVE
                if ncopy[0] % 2 == 0:
                    nc.scalar.copy(out=dst, in_=src)
                else:
                    nc.vector.tensor_copy(out=dst, in_=src)
                ncopy[0] += 1

            # ---- replicated phase A, layer 0: xl0 for ALL slots ----
            for s0 in range(0, NT, SA):
                w_ = min(SA, NT - s0)
                xs_t = stpool.tile([P, SA * P], f16, tag="xstrip")
                nc.sync.dma_start(out=xs_t[:, :w_ * P],
                                  in_=inp["x0T"][:, s0 * P:(s0 + w_) * P])
                ps = psU.tile([P, SA, P], f32, space="PSUM", tag="psu")
                for j in range(w_):
                    nc.tensor.matmul(
                        out=ps[:, j, :], lhsT=xs_t[:, j * P:(j + 1) * P],
                        rhs=wlr_t[0][:, :P], start=True, stop=True)
                stg = stpool.tile([P, SA, P], f16, tag="a0stg")
                psum_copy(stg[:, :w_, :], ps[:, :w_, :])
                nc.sync.dma_start(
                    out=xl_full[0][s0 * P:(s0 + w_) * P, :].rearrange(
                        "(t p) f -> p t f", p=P),
                    in_=stg[:, :w_, :])
            # layer-0 xr for own tiles
            for s0 in range(0, NTC, SA):
                w_ = min(SA, NTC - s0)
                xs_t = stpool.tile([P, SA * P], f16, tag="xstrip")
                nc.sync.dma_start(out=xs_t[:, :w_ * P],
                                  in_=inp["xr0T"][:, s0 * P:(s0 + w_) * P])
                ps = psU.tile([P, SA, P], f32, space="PSUM", tag="psu")
                for j in range(w_):
                    nc.tensor.matmul(
                        out=ps[:, j, :], lhsT=xs_t[:, j * P:(j + 1) * P],
                        rhs=wlr_t[0][:, 128:256], start=True, stop=True)
                psum_copy(xr_sb[:, s0:s0 + w_, :], ps[:, :w_, :])

            # ---- layers ----
            for l in range(3):
                Hh = H_L[l]
                W = W_L[l]
                CW = W // Hh
                att_l = att_t[l]

                for tt0, sz in CHUNKS:
                    for gi, g in _chunk_groups(tt0, sz):
                        info = ginfo[gi]
                        lo_list, hi_list = info["lo"], info["hi"]
                        nch_lo, nch_hi = len(lo_list), len(hi_list)
                        nch2 = nch_lo + nch_hi
                        col0 = info["oo"]

                        ilo = smpool.tile([P, GROUP * MCH * 8], i16, tag="ilo")
                        nc.sync.dma_start(
                            out=ilo[:, :nch_lo * 8],
                            in_=inp["xlidxlo"][
                                :, info["io_lo"]:info["io_lo"] + nch_lo * 8])
                        ihi = smpool.tile([P, GROUP * MCH * 8], i16, tag="ihi")
                        nc.sync.dma_start(
                            out=ihi[:, :nch_hi * 8],
                            in_=inp["xlidxhi"][
                                :, info["io_hi"]:info["io_hi"] + nch_hi * 8])
                        ohT_t = epool.tile([P, 2 * GROUP * MCH, P], f8,
                                           tag="ohT")
                        nc.sync.dma_start(
                            out=ohT_t[:, :nch2, :],
                            in_=inp["ohT"][:, col0:col0 + nch2, :])
                        oh_t = epool.tile([P, 2 * GROUP * MCH, P], f8,
                                          tag="oh")
                        nc.sync.dma_start(
                            out=oh_t[:, :nch2, :],
                            in_=inp["oh"][:, col0:col0 + nch2, :])

                        # gather calls (<= GMAX idxs each) per stream;
                        # GMAX must stay <= half the SWDGE ring (2048 descs)
                        GMAX = 1024
                        def gathers(st, in_ap, idx_t, nch_s):
                            k = 0
                            while k < nch_s * P:
                                n = min(GMAX, nch_s * P - k)
                                nc.gpsimd.dma_gather(
                                    out_ap=st[:, k // P:(k + n) // P, :],
                                    in_ap=in_ap,
                                    idxs_ap=idx_t[:, k // 16:(k + n) // 16],
                                    num_idxs=n, num_idxs_reg=n, elem_size=P)
                                k += n
                        st_lo = epool.tile([P, GROUP * MCH, P], f16,
                                           tag="xlglo")
                        gathers(st_lo, xl_full[l][0:HALF, :], ilo, nch_lo)
                        st_hi = epool.tile([P, GROUP * MCH, P], f16,
                                           tag="xlghi")
                        gathers(st_hi, xl_full[l][HALF:NP_, :], ihi, nch_hi)

                        # u = xr[tloc] + xl_src  (PSUM), leaky-relu -> L
                        L_t = epool.tile([P, 2 * GROUP * MCH, P], f16, tag="L")
                        for c0, xt, clist in ((0, st_lo, lo_list),
                                              (nch_lo, st_hi, hi_list)):
                            for b0 in range(0, len(clist), BATCH):
                                nb = min(BATCH, len(clist) - b0)
                                psu = psU.tile([P, BATCH, P], f32,
                                               space="PSUM", tag="psu")
                                for k in range(nb):
                                    cc = b0 + k
                                    nc.tensor.matmul(
                                        out=psu[:, k, :W],
                                        lhsT=ohT_t[:, c0 + cc, :],
                                        rhs=xr_sb[:, clist[cc], :W],
                                        start=True, stop=False)
                                    nc.tensor.matmul(
                                        out=psu[:, k, :W], lhsT=ident_t[:],
                                        rhs=xt[:, cc, :W],
                                        start=False, stop=True)
                                nc.scalar.activation(
                                    out=L_t[:, c0 + b0:c0 + b0 + nb, :W],
                                    in_=psu[:, :nb, :W],
                                    func=mybir.ActivationFunctionType.Prelu,
                                    alpha=NEG)

                        # scores: L *= att ; tree-reduce over cw
                        nc.vector.tensor_tensor(
                            out=L_t[:, :nch2, :W], in0=L_t[:, :nch2, :W],
                            in1=att_l[:, :W].unsqueeze(1).broadcast_to(
                                [P, nch2, W]),
                            op=mybir.AluOpType.mult)
                        w_t = epool.tile([P, 2 * GROUP * MCH, P + 4], f16,
                                         tag="w")
                        Lv = L_t[:, :nch2, :W].rearrange(
                            "p c (w h) -> p c w h", h=Hh)
                        # tree halves in place into L_t's low columns (L is
                        # dead after the att-mult; keeps w_t alias-free)
                        tv = L_t[:, :nch2, :W // 2].rearrange(
                            "p c (w h) -> p c w h", h=Hh)
                        half = CW // 2
                        nc.vector.tensor_tensor(
                            out=tv[:, :, :half, :], in0=Lv[:, :, :half, :],
                            in1=Lv[:, :, half:, :], op=mybir.AluOpType.add)
                        while half > 1:
                            q = half // 2
                            nc.vector.tensor_tensor(
                                out=tv[:, :, :q, :], in0=tv[:, :, :q, :],
                                in1=tv[:, :, q:half, :], op=mybir.AluOpType.add)
                            half = q
                        # w values and alpha
                        nc.scalar.activation(
                            out=w_t[:, :nch2, W:W + Hh], in_=tv[:, :nch2, 0, :],
                            func=mybir.ActivationFunctionType.Exp)
                        a_b = w_t[:, :nch2, W:W + Hh].unsqueeze(2).broadcast_to(
                            [P, nch2, CW, Hh])
                        for c0, xt, clist in ((0, st_lo, lo_list),
                                              (nch_lo, st_hi, hi_list)):
                            ns = len(clist)
                            nc.vector.tensor_tensor(
                                out=w_t[:, c0:c0 + ns, :W].rearrange(
                                    "p c (w h) -> p c w h", h=Hh),
                                in0=xt[:, :ns, :W].rearrange(
                                    "p c (w h) -> p c w h", h=Hh),
                                in1=a_b[:, c0:c0 + ns],
                                op=mybir.AluOpType.mult)

                        if dbg and l == 0 and gi == 0:
                            nc.sync.dma_start(out=dbg["st0"][:, :nch_lo, :],
                                              in_=st_lo[:, :nch_lo, :])
                            nc.sync.dma_start(out=dbg["L0"][:, :nch2, :],
                                              in_=L_t[:, :nch2, :])
                            nc.sync.dma_start(out=dbg["w0"][:, :nch2, :],
                                              in_=w_t[:, :nch2, :])

                        # scatter per tile
                        for tt_ in range(g):
                            t = gi + tt_
                            cids = ([c0 for c0, tt in enumerate(lo_list)
                                     if tt == t]
                                    + [nch_lo + c0
                                       for c0, tt in enumerate(hi_list)
                                       if tt == t])
                            ps = psS.tile([P, P + 4], f32, space="PSUM",
                                          tag="pss")
                            for cix, cid in enumerate(cids):
                                nc.tensor.matmul(
                                    out=ps[:, :W + Hh],
                                    lhsT=oh_t[:, cid, :],
                                    rhs=w_t[:, cid, :W + Hh],
                                    start=(cix == 0),
                                    stop=(cix == len(cids) - 1))
                            rec = smpool.tile([P, 4], f32, tag="rec")
                            nc.vector.reciprocal(out=rec[:, :Hh],
                                                 in_=ps[:, W:W + Hh])
                            t1 = smpool.tile([P, P], f16, tag="t1")
                            nc.vector.tensor_tensor(
                                out=t1[:, :W].rearrange(
                                    "p (w h) -> p w h", h=Hh),
                                in0=ps[:, :W].rearrange(
                                    "p (w h) -> p w h", h=Hh),
                                in1=rec[:, :Hh].unsqueeze(1).broadcast_to(
                                    [P, CW, Hh]),
                                op=mybir.AluOpType.mult)
                            if l < 2:
                                nc.scalar.activation(
                                    out=stg_xn[:, t, :], in_=t1[:],
                                    func=mybir.ActivationFunctionType.Prelu,
                                    alpha=NEG)
                            else:
                                xnm = smpool.tile([P, G_GRAPHS], f16,
                                                  tag="xnm2")
                                nc.scalar.activation(
                                    out=xnm[:], in_=t1[:, :G_GRAPHS],
                                    func=mybir.ActivationFunctionType.Prelu,
                                    alpha=NEG)
                                nc.tensor.matmul(
                                    out=pool_psum[:],
                                    lhsT=pool_t[:, t, :], rhs=xnm[:],
                                    start=(t == 0), stop=(t == NTC - 1))

                    # ---- interleaved phase A(l+1) + chunk AllGather ----
                    if l < 2:
                        Wn = W_L[l + 1]
                        nc.sync.dma_start(
                            out=xn_own[tt0 * P:(tt0 + sz) * P, :].rearrange(
                                "(t p) f -> p t f", p=P),
                            in_=stg_xn[:, tt0:tt0 + sz, :])
                        for j0 in range(tt0, tt0 + sz, 2):
                            w_ = min(2, tt0 + sz - j0)
                            xs_t = stpool.tile([P, 2 * P], f16, tag="xstrip2")
                            nc.sync.dma_start_transpose(
                                out=xs_t[:, :w_ * P],
                                in_=xn_own[j0 * P:(j0 + w_) * P, :])
                            ps = psU.tile([P, SA, P], f32, space="PSUM",
                                          tag="psu")
                            for j in range(w_):
                                nc.tensor.matmul(
                                    out=ps[:, j, :Wn],
                                    lhsT=xs_t[:, j * P:(j + 1) * P],
                                    rhs=wlr_t[l + 1][:, :Wn],
                                    start=True, stop=True)
                                nc.tensor.matmul(
                                    out=ps[:, 4 + j, :Wn],
                                    lhsT=xs_t[:, j * P:(j + 1) * P],
                                    rhs=wlr_t[l + 1][:, 128:128 + Wn],
                                    start=True, stop=True)
                            psum_copy(stg_xl[:, j0:j0 + w_, :Wn],
                                      ps[:, :w_, :Wn])
                            psum_copy(xr_sb[:, j0:j0 + w_, :Wn],
                                      ps[:, 4:4 + w_, :Wn])
                        Ws = xl_own[l + 1].shape[1]
                        nc.sync.dma_start(
                            out=xl_own[l + 1][tt0 * P:(tt0 + sz) * P, :]
                            .rearrange("(t p) f -> p t f", p=P),
                            in_=stg_xl[:, tt0:tt0 + sz, :Ws])
                        s0 = CORES * tt0 * P
                        s1 = CORES * (tt0 + sz) * P
                        if Wn == P or os.environ.get("GAT_L2FULL"):
                            nc.gpsimd.collective_compute(
                                "AllGather", mybir.AluOpType.bypass,
                                replica_groups=[list(range(CORES))],
                                ins=[xl_own[l + 1][tt0 * P:(tt0 + sz) * P, :]],
                                outs=[xl_full[l + 1][s0:s1, :]])
                        else:
                            nc.gpsimd.collective_compute(
                                "AllGather", mybir.AluOpType.bypass,
                                replica_groups=[list(range(CORES))],
                                ins=[xl_own[l + 1][tt0 * P:(tt0 + sz) * P, :]],
                                outs=[xl2c[s0:s1, :]])
                            nc.sync.dma_start(
                                out=xl_full[l + 1][s0:s1, :Wn],
                                in_=xl2c[s0:s1, :])

                if dbg and l == 0:
                    nc.sync.dma_start(out=dbg["xl0"][:], in_=xl_full[0][:])
                    nc.sync.dma_start(out=dbg["xn0"][:], in_=xn_own[:])
                if dbg and l == 1:
                    nc.sync.dma_start(out=dbg["xl1"][:], in_=xl_full[1][:])

            pool_sb = smpool.tile([G_GRAPHS, G_GRAPHS], f32, tag="poolsb")
            nc.vector.tensor_copy(out=pool_sb[:], in_=pool_psum[:])
            nc.sync.dma_start(out=pooled[:], in_=pool_sb[:])

    nc.finalize()
    return nc


def kernel(**inputs):
    x = np.asarray(inputs["x"])
    edge_index = np.asarray(inputs["edge_index"])
    batch = np.asarray(inputs["batch"])
    params = []
    for l in range(3):
        params.append((np.asarray(inputs[f"Wl{l}"]),
                       np.asarray(inputs[f"Wr{l}"]),
                       np.asarray(inputs[f"att{l}"])))
        b = np.asarray(inputs[f"b{l}"])
        assert np.all(b == 0), "nonzero bias not supported"

    meta, in_maps = _preprocess(x, edge_index, batch, params)

    key = ("nc", meta["chs_lo"], meta["chs_hi"])
    if key not in _CACHE:
        _CACHE[key] = _build(meta)
    nc = _CACHE[key]

    try:
        res = run_bass_kernel_spmd(
            nc, in_maps, core_ids=list(range(CORES)),
            trace=bool(os.environ.get("GAT_TRACE")))
    except ModuleNotFoundError:
        res = run_bass_kernel_spmd(nc, in_maps, core_ids=list(range(CORES)))
    kernel._last_result = res

    pooled = np.zeros((G_GRAPHS, G_GRAPHS), np.float64)
    for c in range(CORES):
        pooled += res.results[c]["pooled"].astype(np.float64)
    cnt = np.bincount(batch, minlength=G_GRAPHS).astype(np.float64)
    out = pooled / np.maximum(cnt, 1.0)[:, None]
    return out.astype(np.float32)
